# revision 43
# baseline (speedup 1.0000x reference)
"""Trainium2 Bass kernel for nn_CPCLoss (self-contained).

Strategy (8 NeuronCores, full inputs in / full output out):
  NEFF-A, SPMD on 8 cores — core k = (batch b=k//4, quarter q=k%4).
  Each core:
    * pixel phase on its row-block shard cam[b, :, q*112:(q+1)*112, :]:
      top1 via pairwise tree-max, one-hot argmax via (v >= top1), second
      via tree-max of (v - onehot), keep-gate from thresholds/margin,
      then per-class A-partials A_c = Wr^T @ onehot_c @ Wc through two
      transpose-free PE matmul stages (contract rows first, then cols).
    * exact global per-class top-256 over the FULL image for its 3-class
      slice cam[b, 3q:3q+3, :, :] via the gpsimd topk instruction; the
      top-32 pixel indices are converted to (h, w) on-device.
  Host only reshapes/concats partials (no arithmetic).
  NEFF-B, 1 core — sums the 4 row-block A-partials, computes counts,
  builds the top-25 gather matrix G from the shipped (h, w) candidates
  (static rank weights - no merge needed since candidates are already
  global top-32 sorted), selects coef = count==0 ? G : A/count, computes
  fsm^T = coef @ fmap^T via PE, and runs the 2-step EMA memory-bank scan
  (specialized to the spec's feature_contrast == 0 initial bank) to emit
  the scalar loss.
"""
import math
import os
import sys

os.environ.setdefault("MYCRO_LOCAL_CACHE", "1")
if "/opt/trn_rl_repo" not in sys.path:
    sys.path.insert(0, "/opt/trn_rl_repo")

from contextlib import ExitStack

import numpy as np

from concourse import bacc, bass_isa, mybir, tile
from concourse.bass_utils import run_bass_kernel_spmd

f32 = mybir.dt.float32
f32r = mybir.dt.float32r
bf16 = mybir.dt.bfloat16
i32 = mybir.dt.int32
u32 = mybir.dt.uint32
ALU = mybir.AluOpType
AFT = mybir.ActivationFunctionType
AX = mybir.AxisListType

B, C, D = 2, 20, 256
H = W = 448
FH = FW = 28
K_TOP = 25
NBLK = 4
RB = H // NBLK            # 112
NPIX = RB * W             # 50176 (per row-block shard)
FPIX = H * W              # 200704 (full image)
MARGIN = 0.3
NC = 32                   # candidates kept per (b, class)


def _make_w1d():
    scale = FH / H
    w = np.zeros((H, FH), dtype=np.float64)
    for x in range(H):
        s = (x + 0.5) * scale - 0.5
        i0 = int(np.floor(s))
        f = s - i0
        for i, wt in ((i0, 1.0 - f), (i0 + 1, f)):
            if 0 <= i < FH:
                w[x, i] += wt
        w[x] /= w[x].sum()
    return w.astype(np.float32)


W1D = _make_w1d()


def _emit_topk(nc, out_ap, in_ap, tokens, n):
    g = nc.gpsimd
    return g.add_instruction(bass_isa.InstTopk(
        name=f"I-{nc.next_id()}",
        ins=[g.lower_ap(in_ap, for_isa=True)],
        outs=[g.lower_ap(out_ap, for_isa=True)],
        _tokens=tokens, _n=n, _k=256))


def _tree_max(nc, pool, src, nplane, width, name):
    """Pairwise tree max over `nplane` planes of `width` cols in src
    ([112, nplane*width]); returns a [112, width] tile. Emits ~log2 ops."""
    nv = nc.vector
    cur = src
    cnt = nplane
    lvl = 0
    while cnt > 1:
        half = cnt // 2
        odd = cnt - 2 * half
        dst = pool.tile([RB, (half + odd) * width], f32, name=f"{name}_l{lvl}")
        nv.tensor_tensor(out=dst[:, 0:half * width],
                         in0=cur[:, 0:half * width],
                         in1=cur[:, half * width:2 * half * width], op=ALU.max)
        if odd:
            nv.tensor_copy(dst[:, half * width:(half + 1) * width],
                           cur[:, 2 * half * width:(2 * half + 1) * width])
        cur = dst[:]
        cnt = half + odd
        lvl += 1
    return cur


# --------------------------------------------------------------------------
# NEFF-A
# --------------------------------------------------------------------------

def _build_a(hig, low, bg, CP):
    nc = bacc.Bacc("TRN2", target_bir_lowering=False, debug=False, num_devices=8)
    TOKS = (CP + 3) // 4  # classes handled by this core's global topk
    T4 = 4 * TOKS         # round-1 tokens: (class, row-block)
    grps = []             # round-1 token-tile sizes (<= 8 tokens each)
    t = T4
    while t > 0:
        grps.append(min(8, t))
        t -= grps[-1]

    camv = nc.dram_tensor("camv", [CP, NPIX], f32, kind="ExternalInput").ap()
    camt = [nc.dram_tensor(f"camt{i}", [16 * g, NPIX // 16], f32,
                           kind="ExternalInput").ap() for i, g in enumerate(grps)]
    bbs1 = [nc.dram_tensor(f"bbs1_{i}", [16 * g, 1], f32,
                           kind="ExternalInput").ap() for i, g in enumerate(grps)]
    selm = nc.dram_tensor("selm", [TOKS, 16 * 4 * TOKS], f32,
                          kind="ExternalInput").ap()
    idna = nc.dram_tensor("idna", [128, 64], f32, kind="ExternalInput").ap()
    wrt = nc.dram_tensor("wrt", [RB, 28], f32, kind="ExternalInput").ap()
    wct = nc.dram_tensor("wct", [RB, 4 * 28], f32, kind="ExternalInput").ap()

    o_a = nc.dram_tensor("o_a", [28, CP * 28], f32, kind="ExternalOutput").ap()
    o_ch = nc.dram_tensor("o_ch", [16 * TOKS, 2], f32, kind="ExternalOutput").ap()
    o_cw = nc.dram_tensor("o_cw", [16 * TOKS, 2], f32, kind="ExternalOutput").ap()

    thmax = float(max(hig, low, bg))

    with tile.TileContext(nc) as tc, ExitStack() as ctx:
        pool = ctx.enter_context(tc.tile_pool(name="p", bufs=1))
        psum = ctx.enter_context(tc.tile_pool(name="ps", bufs=1, space="PSUM"))
        nv = nc.vector
        ns = nc.scalar

        # DMA order matters: VP first (it gates the long DVE chain), then the
        # topk inputs, then the small/late tensors.
        VP = pool.tile([RB, CP * W], f32)
        CPA = CP // 2
        nc.sync.dma_start(VP[:, 0:CPA * W],
                          camv[0:CPA, :].rearrange("c (r w) -> r c w", w=W))
        nc.sync.dma_start(VP[:, CPA * W:CP * W],
                          camv[CPA:CP, :].rearrange("c (r w) -> r c w", w=W))
        VT = [pool.tile([16 * g, NPIX // 16], f32, name=f"vt{i}")
              for i, g in enumerate(grps)]
        for i in range(len(grps)):
            nc.scalar.dma_start(VT[i][:], camt[i])
        WR = pool.tile([RB, 28], f32); nc.gpsimd.dma_start(WR[:], wrt)
        WC = pool.tile([RB, 4 * 28], f32); nc.gpsimd.dma_start(WC[:], wct)
        BBS1 = [pool.tile([16 * g, 1], f32, name=f"bbs1_{i}")
                for i, g in enumerate(grps)]
        for i in range(len(grps)):
            nc.scalar.dma_start(BBS1[i][:], bbs1[i])
        SELM = pool.tile([TOKS, 16 * T4], f32)
        nc.scalar.dma_start(SELM[:], selm)
        IDN64 = pool.tile([128, 64], f32)
        nc.scalar.dma_start(IDN64[:], idna)

        # ---- round 1 topk first in the Pool queue (nothing may block it) ----
        TK1 = [pool.tile([16 * g, 32], u32, name=f"tk1_{i}")
               for i, g in enumerate(grps)]
        for i, g in enumerate(grps):
            _emit_topk(nc, TK1[i][:], VT[i][:], tokens=g, n=NPIX)

        # ---- pixel phase (the long DVE chain goes first in the DVE queue) ----
        # two sub-trees so the first starts as soon as half of VP is loaded
        T1a = _tree_max(nc, pool, VP[:, 0:CPA * W], CPA, W, "t1a")
        T1b = _tree_max(nc, pool, VP[:, CPA * W:CP * W], CP - CPA, W, "t1b")
        T1t = pool.tile([RB, W], f32)
        nv.tensor_tensor(out=T1t[:], in0=T1a, in1=T1b, op=ALU.max)
        T1 = T1t[:]

        GE = pool.tile([RB, CP * W], f32)
        GE_cw = GE[:].rearrange("p (c w) -> p c w", w=W)
        V_cw = VP[:].rearrange("p (c w) -> p c w", w=W)
        T1_b = T1.unsqueeze(1).broadcast_to([RB, CP, W])
        nv.tensor_tensor(out=GE_cw, in0=V_cw, in1=T1_b, op=ALU.is_ge)

        # second max: max over c of (v - onehot);  (in-place on VP)
        nv.tensor_tensor(out=VP[:], in0=VP[:], in1=GE[:], op=ALU.subtract)
        SC = _tree_max(nc, pool, VP[:], CP, W, "sc")

        # keep iff top1 >= thmax and margin >= 0.3
        KG = pool.tile([RB, W], f32)
        nv.tensor_scalar(out=KG[:], in0=T1, scalar1=thmax, scalar2=None, op0=ALU.is_ge)
        MG = pool.tile([RB, W], f32)
        nv.tensor_tensor(out=MG[:], in0=T1, in1=SC, op=ALU.subtract)
        nv.tensor_scalar(out=MG[:], in0=MG[:], scalar1=MARGIN, scalar2=None, op0=ALU.is_ge)
        nv.tensor_tensor(out=KG[:], in0=KG[:], in1=MG[:], op=ALU.mult)

        # ---- per-class masks + A partials (two transpose-free stages) ----
        KEEP = pool.tile([RB, CP * W], f32)
        KG_b3 = KG[:].unsqueeze(1)
        Usb = pool.tile([RB, CP * 4 * 28], f32)
        ns1 = 0
        s1ps = []
        s1dst = []

        def flush_s1():
            nonlocal s1ps, s1dst
            for ps_t, (s0, cnt) in zip(s1ps, s1dst):
                ns.copy(Usb[:, s0 * 28:(s0 + cnt) * 28], ps_t[:, 0:cnt * 28])
            s1ps, s1dst = [], []

        cq, rem = divmod(CP, 4)
        chunks = [cq + (1 if g < rem else 0) for g in range(4)]
        c_at = 0
        for g in range(4):
            CQ = chunks[g]
            if CQ == 0:
                continue
            sl = slice(c_at * W, (c_at + CQ) * W)
            nv.tensor_tensor(out=KEEP[:, sl].rearrange("p (c w) -> p c w", w=W),
                             in0=GE[:, sl].rearrange("p (c w) -> p c w", w=W),
                             in1=KG_b3.broadcast_to([RB, CQ, W]), op=ALU.mult)
            for cc in range(CQ):
                c = c_at + cc
                for u in range(4):
                    if ns1 % 5 == 0:
                        s1ps.append(psum.tile([RB, 140], f32, tag="s1", bufs=3,
                                              name=f"s1_{ns1}"))
                        s1dst.append((c * 4 + u, 0))
                    off = (ns1 % 5) * 28
                    nc.tensor.matmul(
                        s1ps[-1][:, off:off + 28],
                        lhsT=KEEP[:, c * W + u * RB:c * W + (u + 1) * RB],
                        rhs=WR[:], start=True, stop=True)
                    s0, cnt = s1dst[-1]
                    s1dst[-1] = (s0, cnt + 1)
                    ns1 += 1
                    if ns1 % 5 == 0:
                        flush_s1()
            c_at += CQ
        flush_s1()

        # stage 2: A_c[i,j] = sum_u U_cu^T @ Wc_u   (5 classes per PSUM bank)
        ngrp = (CP + 4) // 5
        Asb = pool.tile([28, CP * 28], f32)
        for grp in range(ngrp):
            c0 = grp * 5
            c1 = min(c0 + 5, CP)
            aps = psum.tile([28, (c1 - c0) * 28], f32, tag="s2", bufs=2,
                            name=f"s2_{grp}")
            for c in range(c0, c1):
                off = (c - c0) * 28
                for u in range(4):
                    nc.tensor.matmul(
                        aps[:, off:off + 28],
                        lhsT=Usb[:, (c * 4 + u) * 28:(c * 4 + u + 1) * 28],
                        rhs=WC[:, u * 28:(u + 1) * 28],
                        start=(u == 0), stop=(u == 3))
            ns.copy(Asb[:, c0 * 28:c1 * 28], aps[:])
        nc.sync.dma_start(o_a, Asb[:])

        # ---- round 2: global top-25 per class ----
        # Root ops are anchored on KEEP's last column (a real data dep with
        # zero effect) so the greedy in-order scheduler cannot interleave
        # them into the pixel chain: they only become ready after it.
        tc.cur_priority += 100000
        starts = []
        s = 0
        for g in grps:
            starts.append(s)
            s += g
        anc_last = CP * W - 1
        CROWS = pool.tile([TOKS, 128], f32)
        loc = []
        for ct in range(TOKS):
            i = next(j for j, st in enumerate(starts)
                     if st <= 4 * ct < st + grps[j])
            r0 = (4 * ct - starts[i]) * 16
            loc.append((i, r0))
            for k in range(4):
                rr = r0 + 16 * k + 14
                nc.scalar.dma_start(
                    CROWS[ct:ct + 1, 32 * k:32 * k + 32],
                    TK1[i][rr:rr + 2, 0:16].bitcast(f32))
        MV = pool.tile([TOKS, 32], f32)
        CVa = pool.tile([TOKS, 128], f32)
        nv.scalar_tensor_tensor(
            out=CVa[:], in0=KEEP[0:TOKS, anc_last:anc_last + 1]
            .broadcast_to([TOKS, 128]), scalar=0.0, in1=CROWS[:],
            op0=ALU.mult, op1=ALU.add)
        for r in range(4):
            nv.max(out=MV[:, r * 8:(r + 1) * 8], in_=CVa[:])
            nv.match_replace(out=CVa[:], in_to_replace=MV[:, r * 8:(r + 1) * 8],
                             in_values=CVa[:], imm_value=-1.0)
        # sanitize: absent classes (all-zero) must select nothing
        THZ = pool.tile([TOKS, 1], f32)
        nv.tensor_scalar(out=THZ[:], in0=MV[:, 24:25], scalar1=0.0, scalar2=None,
                         op0=ALU.is_le)
        THS = pool.tile([TOKS, 1], f32)
        nv.scalar_tensor_tensor(out=THS[:], in0=THZ[:], scalar=1e30,
                                in1=MV[:, 24:25], op0=ALU.mult, op1=ALU.add)
        # group-level mask + masked-index build (all operands base-0)
        MSKg, MI1g = {}, {}
        for i, g in enumerate(grps):
            THRps = psum.tile([16 * g, 1], f32, tag="thr", bufs=1, name=f"thr{i}")
            nc.tensor.matmul(THRps[:], lhsT=SELM[:, 16 * starts[i]:16 * (starts[i] + g)],
                             rhs=THS[:], start=True, stop=True)
            MSK1 = pool.tile([16 * g, 16], f32, name=f"msk1_{i}")
            nv.tensor_scalar(out=MSK1[:], in0=TK1[i][:, 0:16].bitcast(f32),
                             scalar1=THRps[:], scalar2=None, op0=ALU.is_ge)
            GIC = pool.tile([16 * g, 16], f32, name=f"gic{i}")
            nv.scalar_tensor_tensor(out=GIC[:], in0=MSK1[:], scalar=0.0,
                                    in1=TK1[i][:, 16:32], op0=ALU.mult, op1=ALU.add)
            nv.tensor_scalar(out=GIC[:], in0=GIC[:], scalar1=BBS1[i][:],
                             scalar2=None, op0=ALU.add)
            # masked idx: sel ? gidx : -1   (gidx+1 > 0, so mask*(g+1)-1)
            MI1 = pool.tile([16 * g, 16], f32, name=f"mi1_{i}")
            nv.tensor_scalar(out=MI1[:], in0=GIC[:], scalar1=1.0, scalar2=None,
                             op0=ALU.add)
            nv.tensor_tensor(out=MI1[:], in0=MI1[:], in1=MSK1[:], op=ALU.mult)
            nv.tensor_scalar(out=MI1[:], in0=MI1[:], scalar1=1.0, scalar2=None,
                             op0=ALU.subtract)
            MSKg[i], MI1g[i] = MSK1, MI1
        for ct in range(TOKS):
            i, r0 = loc[ct]
            mps = psum.tile([16, 64], f32, tag="r2ps", bufs=2, name=f"mps{ct}")
            nc.tensor.transpose(mps[:], MI1g[i][r0:r0 + 64, :], IDN64[r0:r0 + 64, :])
            MIc = pool.tile([16, 64], f32, name=f"mic{ct}")
            ns.copy(MIc[:], mps[:])
            CIc = pool.tile([16, 2], f32, name=f"cic{ct}")
            NFc = pool.tile([1, 1], u32, name=f"nfc{ct}")
            nc.gpsimd.sparse_gather(CIc[:], MIc[:], num_found=NFc[:])
            # convert to (h, w); -1 pads land harmlessly (weight 0 in B)
            HI = pool.tile([16, 2], i32, name=f"hi{ct}")
            nv.tensor_scalar(out=HI[:], in0=CIc[:], scalar1=1.0 / 448.0,
                             scalar2=(-0.5 + 1.0 / 1024.0), op0=ALU.mult,
                             op1=ALU.add)
            HF = pool.tile([16, 2], f32, name=f"hf{ct}")
            nv.tensor_copy(HF[:], HI[:])
            WF = pool.tile([16, 2], f32, name=f"wf{ct}")
            nv.scalar_tensor_tensor(out=WF[:], in0=HF[:], scalar=-448.0,
                                    in1=CIc[:], op0=ALU.mult, op1=ALU.add)
            nc.scalar.dma_start(o_ch[16 * ct:16 * (ct + 1), :], HF[:])
            nc.scalar.dma_start(o_cw[16 * ct:16 * (ct + 1), :], WF[:])

    nc.compile()
    return nc


# --------------------------------------------------------------------------
# NEFF-B
# --------------------------------------------------------------------------

def _build_b(CP):
    nc = bacc.Bacc("TRN2", target_bir_lowering=False, debug=False, num_devices=1)
    P = B * C  # 40 (b,c) pairs, b-major

    ain = nc.dram_tensor("ain", [P, NBLK * 784], f32, kind="ExternalInput").ap()
    cdh = nc.dram_tensor("cdh", [P, NC], f32, kind="ExternalInput").ap()
    cdw = nc.dram_tensor("cdw", [P, NC], f32, kind="ExternalInput").ap()
    rnk = nc.dram_tensor("rnk", [P, NC], f32, kind="ExternalInput").ap()
    fmi = nc.dram_tensor("fmi", [112, 7 * B * D], f32, kind="ExternalInput").ap()
    prj = nc.dram_tensor("prj", [128, 2 * C], f32, kind="ExternalInput").ap()
    lab = nc.dram_tensor("lab", [P, 1], f32, kind="ExternalInput").ap()
    lab2 = nc.dram_tensor("lab2", [C, B], f32, kind="ExternalInput").ap()
    lrep = nc.dram_tensor("lrep", [128, P], f32, kind="ExternalInput").ap()
    eye = nc.dram_tensor("eye", [C, C], f32, kind="ExternalInput").ap()
    i28 = nc.dram_tensor("i28", [128, 28], f32, kind="ExternalInput").ap()
    mmb = nc.dram_tensor("mmb", [128, 76], f32, kind="ExternalInput").ap()
    idn = nc.dram_tensor("idn", [128, 128], f32, kind="ExternalInput").ap()
    onr = nc.dram_tensor("onr", [1, 128], f32, kind="ExternalInput").ap()

    o_loss = nc.dram_tensor("o_loss", [1, 1], f32, kind="ExternalOutput").ap()

    LN_EPS = 1e-5
    ALPHA = float(-(math.log(LN_EPS) - math.log1p(-LN_EPS)) / (C * C))
    BETA = float(-math.log1p(-LN_EPS))

    with tile.TileContext(nc) as tc, ExitStack() as ctx:
        pool = ctx.enter_context(tc.tile_pool(name="p", bufs=1))
        psum = ctx.enter_context(tc.tile_pool(name="ps", bufs=1, space="PSUM"))
        nv = nc.vector
        ns = nc.scalar

        CHW = pool.tile([P, 2 * NC], f32)
        nc.sync.dma_start(CHW[:, 0:NC], cdh)
        nc.sync.dma_start(CHW[:, NC:2 * NC], cdw)
        RNK = pool.tile([P, NC], f32); nc.sync.dma_start(RNK[:], rnk)
        AIN = pool.tile([P, NBLK * 784], f32); nc.scalar.dma_start(AIN[:], ain)
        FM = pool.tile([112, 7 * B * D], f32); nc.scalar.dma_start(FM[:], fmi)
        PJT = pool.tile([128, 2 * C], f32); nc.scalar.dma_start(PJT[:], prj)
        LAB = pool.tile([P, 1], f32); nc.scalar.dma_start(LAB[:], lab)
        LAB2 = pool.tile([C, B], f32); nc.scalar.dma_start(LAB2[:], lab2)
        LREP = pool.tile([128, P], f32); nc.gpsimd.dma_start(LREP[:], lrep)
        EYE = pool.tile([C, C], f32); nc.gpsimd.dma_start(EYE[:], eye)
        I28 = pool.tile([128, 28], f32); nc.gpsimd.dma_start(I28[:], i28)
        MMB = pool.tile([128, 76], f32); nc.gpsimd.dma_start(MMB[:], mmb)
        MMBH = pool.tile([128, 76], bf16)
        nv.tensor_copy(MMBH[:], MMB[:])
        IDN = pool.tile([128, 128], f32); nc.gpsimd.dma_start(IDN[:], idn)
        ONR = pool.tile([1, 128], f32); nc.gpsimd.dma_start(ONR[:], onr)

        def ts(dst, src, s1, s2, op0, op1=None):
            nv.tensor_scalar(out=dst, in0=src, scalar1=s1, scalar2=s2, op0=op0,
                             **({"op1": op1} if op1 is not None else {}))

        # ---- interpolation coefficients (written straight into STG) ----
        STG = pool.tile([P, NC * 8], f32)
        STG_v = STG[:].rearrange("p (k a) -> p k a", a=8)

        def sv(idx):
            return STG_v[:, :, idx:idx + 1]

        # combined H|W chain on [P, 2*NC]: u = (x+8.5)/16; fl = floor(u) =
        # rtn((x+0.5)/16); f = u-fl; x0/x1 = clip(fl-1)/clip(fl); w0 = 1-f
        U = pool.tile([P, 2 * NC], f32)
        ts(U[:], CHW[:], 8.5, 1.0 / 16.0, ALU.add, ALU.mult)
        FLI = pool.tile([P, 2 * NC], i32)
        ts(FLI[:], CHW[:], 0.5, 1.0 / 16.0, ALU.add, ALU.mult)
        FLF = pool.tile([P, 2 * NC], f32)
        nv.tensor_copy(FLF[:], FLI[:])
        F = pool.tile([P, 2 * NC], f32)
        nv.tensor_tensor(out=F[:], in0=U[:], in1=FLF[:], op=ALU.subtract)
        X0 = pool.tile([P, 2 * NC], f32)
        ts(X0[:], FLF[:], 1.0, None, ALU.subtract)
        W0 = pool.tile([P, 2 * NC], f32)
        ts(W0[:], F[:], -1.0, 1.0, ALU.mult, ALU.add)
        ts(sv(0), X0[:, 0:NC].unsqueeze(2), 0.0, 27.0, ALU.max, ALU.min)
        ts(sv(1), FLF[:, 0:NC].unsqueeze(2), 0.0, 27.0, ALU.max, ALU.min)
        nv.tensor_copy(sv(2), W0[:, 0:NC].unsqueeze(2))
        nv.tensor_copy(sv(3), F[:, 0:NC].unsqueeze(2))
        ts(sv(4), X0[:, NC:2 * NC].unsqueeze(2), 0.0, 27.0, ALU.max, ALU.min)
        ts(sv(5), FLF[:, NC:2 * NC].unsqueeze(2), 0.0, 27.0, ALU.max, ALU.min)
        nv.tensor_tensor(out=sv(6), in0=W0[:, NC:2 * NC].unsqueeze(2),
                         in1=RNK[:].unsqueeze(2), op=ALU.mult)
        nv.tensor_tensor(out=sv(7), in0=F[:, NC:2 * NC].unsqueeze(2),
                         in1=RNK[:].unsqueeze(2), op=ALU.mult)

        # ---- stage (pair,k) scalars onto partitions: 10 groups of 4 pairs ----
        FLT = pool.tile([128, 80], f32)
        qs = [nc.sync, nc.scalar, nc.gpsimd]
        for g in range(10):
            qs[g % 3].dma_start(
                FLT[:, g * 8:(g + 1) * 8],
                STG[g * 4:(g + 1) * 4, :].rearrange("p (k a) -> p k a", a=8))

        # ---- batched row/col factors + outer products + banded matmuls ----
        I28b = I28[:].unsqueeze(1).broadcast_to([128, 10, 28])

        def fb(idx):
            return FLT[:, idx::8].unsqueeze(2).broadcast_to([128, 10, 28])

        EQ0 = pool.tile([128, 10 * 28], f32)
        EQ0v = EQ0[:].rearrange("p (g a) -> p g a", a=28)
        nv.tensor_tensor(out=EQ0v, in0=I28b, in1=fb(0), op=ALU.is_equal)
        RR = pool.tile([128, 10 * 28], f32)
        RRv = RR[:].rearrange("p (g a) -> p g a", a=28)
        nv.tensor_tensor(out=RRv, in0=EQ0v, in1=fb(2), op=ALU.mult)
        EQ1 = pool.tile([128, 10 * 28], f32)
        EQ1v = EQ1[:].rearrange("p (g a) -> p g a", a=28)
        nv.tensor_tensor(out=EQ1v, in0=I28b, in1=fb(1), op=ALU.is_equal)
        nv.tensor_tensor(out=EQ1v, in0=EQ1v, in1=fb(3), op=ALU.mult)
        nv.tensor_tensor(out=RR[:], in0=RR[:], in1=EQ1[:], op=ALU.add)

        EQ2 = pool.tile([128, 10 * 28], f32)
        EQ2v = EQ2[:].rearrange("p (g a) -> p g a", a=28)
        nv.tensor_tensor(out=EQ2v, in0=I28b, in1=fb(4), op=ALU.is_equal)
        CC = pool.tile([128, 10 * 28], f32)
        CCv = CC[:].rearrange("p (g a) -> p g a", a=28)
        nv.tensor_tensor(out=CCv, in0=EQ2v, in1=fb(6), op=ALU.mult)
        EQ3 = pool.tile([128, 10 * 28], f32)
        EQ3v = EQ3[:].rearrange("p (g a) -> p g a", a=28)
        nv.tensor_tensor(out=EQ3v, in0=I28b, in1=fb(5), op=ALU.is_equal)
        nv.tensor_tensor(out=EQ3v, in0=EQ3v, in1=fb(7), op=ALU.mult)
        nv.tensor_tensor(out=CC[:], in0=CC[:], in1=EQ3[:], op=ALU.add)

        G = pool.tile([P, 784], f32)
        GpsA = psum.tile([P, 392], f32)
        GpsB = psum.tile([P, 392], f32)
        for g in range(10):
            RHS = pool.tile([128, 784], bf16, tag="rhs", bufs=2)
            nv.tensor_tensor(out=RHS[:].rearrange("p (a b) -> p a b", b=28),
                             in0=RR[:, g * 28:(g + 1) * 28].unsqueeze(2)
                             .broadcast_to([128, 28, 28]),
                             in1=CC[:, g * 28:(g + 1) * 28].unsqueeze(1)
                             .broadcast_to([128, 28, 28]),
                             op=ALU.mult)
            lhsT_g = MMBH[:, 36 - 4 * g:76 - 4 * g]
            nc.tensor.matmul(GpsA[:], lhsT=lhsT_g, rhs=RHS[:, 0:392],
                             start=(g == 0), stop=(g == 9))
            nc.tensor.matmul(GpsB[:], lhsT=lhsT_g, rhs=RHS[:, 392:784],
                             start=(g == 0), stop=(g == 9))
        ns.copy(G[:, 0:392], GpsA[:])
        ns.copy(G[:, 392:784], GpsB[:])

        # ---- A sum over 4 row-blocks, counts, coef select ----
        A = pool.tile([P, 784], f32)
        S01 = pool.tile([P, 784], f32)
        nv.tensor_tensor(out=S01[:], in0=AIN[:, 0:784], in1=AIN[:, 784:1568],
                         op=ALU.add)
        nv.tensor_tensor(out=A[:], in0=AIN[:, 1568:2352], in1=AIN[:, 2352:3136],
                         op=ALU.add)
        nv.tensor_tensor(out=A[:], in0=A[:], in1=S01[:], op=ALU.add)
        CNT = pool.tile([P, 1], f32)
        nv.tensor_reduce(out=CNT[:], in_=A[:], axis=AX.X, op=ALU.add)
        ISZ = pool.tile([P, 1], u32)
        ts(ISZ[:], CNT[:], 0.5, None, ALU.is_lt)
        DEN = pool.tile([P, 1], f32)
        ts(DEN[:], CNT[:], 1.0, None, ALU.max)
        RDEN = pool.tile([P, 1], f32)
        nv.reciprocal(RDEN[:], DEN[:])
        AMN = pool.tile([P, 784], f32)
        ts(AMN[:], A[:], RDEN[:], None, ALU.mult)
        COEF = pool.tile([P, 784], f32)
        nv.select(COEF[:], ISZ[:].broadcast_to([P, 784]), G[:], AMN[:])
        ts(COEF[:], COEF[:], LAB[:], None, ALU.mult)

        # ---- coef^T chunks + fsm^T ----
        CT = pool.tile([RB, 7 * P], f32)
        for u in range(7):
            TPS = psum.tile([RB, P], f32, tag="tps", bufs=2)
            nc.tensor.transpose(TPS[:], COEF[:, u * RB:(u + 1) * RB], IDN[:P, :P])
            ns.copy(CT[:, u * P:(u + 1) * P], TPS[:])

        # FSMT[d, (dh, b, c)] chunks: acc over 7 s-chunks
        FSMT = pool.tile([128, 2 * P], f32)
        for dh in range(2):
            for b2 in range(B):
                fps = psum.tile([128, C], f32, tag="fsps", bufs=2)
                for u in range(7):
                    nc.tensor.matmul(
                        fps[:],
                        lhsT=FM[:, u * (B * D) + b2 * D + dh * 128:
                                u * (B * D) + b2 * D + (dh + 1) * 128],
                        rhs=CT[:, u * P + b2 * C:u * P + (b2 + 1) * C],
                        start=(u == 0), stop=(u == 6))
                ns.copy(FSMT[:, dh * P + b2 * C:dh * P + (b2 + 1) * C], fps[:])

        # ---- prescan: fsm norms (both b), softmax/term (both b) ----
        SQ = pool.tile([128, 2 * P], f32)
        nv.tensor_tensor(out=SQ[:], in0=FSMT[:], in1=FSMT[:], op=ALU.mult)
        ONC = pool.tile([128, 1], f32)
        nv.memset(ONC[:], 1.0)
        NNps = psum.tile([1, P], f32, tag="mm", bufs=2)
        for dh in range(2):
            nc.tensor.matmul(NNps[:], lhsT=ONC[:], rhs=SQ[:, dh * P:(dh + 1) * P],
                             start=(dh == 0), stop=(dh == 1))
        NN2 = pool.tile([1, P], f32)
        ts(NN2[:], NNps[:], 1e-24, None, ALU.max)
        NRM = pool.tile([1, P], f32)
        ns.activation(NRM[:], NN2[:], AFT.Sqrt)
        RN = pool.tile([1, P], f32)
        nv.reciprocal(RN[:], NRM[:])
        RNps = psum.tile([128, P], f32, tag="mm", bufs=2)
        nc.tensor.matmul(RNps[:], lhsT=ONR[:], rhs=RN[:], start=True, stop=True)
        FSMNT = pool.tile([128, 2 * P], f32)
        FSMNTv = FSMNT[:].rearrange("d (h p) -> d h p", p=P)
        nv.tensor_tensor(out=FSMNTv, in0=FSMT[:].rearrange("d (h p) -> d h p", p=P),
                         in1=RNps[:].unsqueeze(1).broadcast_to([128, 2, P]),
                         op=ALU.mult)

        # softmax/log terms for both b at once: LOG [C, (b, c')]
        TERM = pool.tile([C, B], f32)
        PR = pool.tile([C, 5], f32)
        for b2 in range(B):
            LOGps = psum.tile([C, C], f32, tag="mm", bufs=2)
            for dh in range(2):
                nc.tensor.matmul(LOGps[:],
                                 lhsT=FSMT[:, dh * P + b2 * C:dh * P + (b2 + 1) * C],
                                 rhs=PJT[:, dh * C:(dh + 1) * C],
                                 start=(dh == 0), stop=(dh == 1))
            MX = pool.tile([C, 1], f32, tag=f"mx{b2}")
            nv.tensor_reduce(out=MX[:], in_=LOGps[:], axis=AX.X, op=ALU.max)
            XT = pool.tile([C, C], f32, tag=f"xt{b2}")
            ts(XT[:], LOGps[:], MX[:], None, ALU.subtract)
            ET = pool.tile([C, C], f32, tag=f"et{b2}")
            ns.activation(ET[:], XT[:], AFT.Exp)
            SM = pool.tile([C, 1], f32, tag=f"sm{b2}")
            nv.tensor_reduce(out=SM[:], in_=ET[:], axis=AX.X, op=ALU.add)
            LGS = pool.tile([C, 1], f32, tag=f"lgs{b2}")
            ns.activation(LGS[:], SM[:], AFT.Ln)
            LGP = pool.tile([C, C], f32, tag=f"lgp{b2}")
            ts(LGP[:], XT[:], LGS[:], -100.0, ALU.subtract, ALU.max)
            SME = pool.tile([C, C], f32, tag=f"sme{b2}")
            nv.tensor_tensor(out=SME[:], in0=SM[:].broadcast_to([C, C]), in1=ET[:],
                             op=ALU.subtract)
            LSME = pool.tile([C, C], f32, tag=f"lsme{b2}")
            ns.activation(LSME[:], SME[:], AFT.Ln)
            L1P = pool.tile([C, C], f32, tag=f"l1p{b2}")
            ts(L1P[:], LSME[:], LGS[:], -100.0, ALU.subtract, ALU.max)
            DD = pool.tile([C, C], f32, tag=f"dd{b2}")
            nv.tensor_tensor(out=DD[:], in0=LGP[:], in1=L1P[:], op=ALU.subtract)
            DDS = pool.tile([C, C], f32, tag=f"dds{b2}")
            DDG = pool.tile([C, 1], f32, tag=f"ddg{b2}")
            nv.tensor_tensor(out=DDS[:], in0=EYE[:], in1=DD[:], op=ALU.mult)
            nv.tensor_reduce(out=DDG[:], in_=DDS[:], axis=AX.X, op=ALU.add)
            RSM = pool.tile([C, 1], f32, tag=f"rsm{b2}")
            nv.tensor_reduce(out=RSM[:], in_=L1P[:], axis=AX.X, op=ALU.add)
            TRM = pool.tile([C, 1], f32, tag=f"trm{b2}")
            nv.tensor_tensor(out=TRM[:], in0=DDG[:], in1=RSM[:], op=ALU.add)
            ts(TERM[:, b2:b2 + 1], TRM[:], -1.0 / C, None, ALU.mult)

        # PR col0 = pres0*term0, col1 = pres0 (iter-0 under fc0 == 0)
        nv.tensor_tensor(out=PR[:, 0:1], in0=TERM[:, 0:1], in1=LAB2[:, 0:1],
                         op=ALU.mult)
        nv.tensor_copy(PR[:, 1:2], LAB2[:, 0:1])

        # ---- iter 1 (fc after iter0 = 0.05 * pres0 * fsm0) ----
        FCNT = pool.tile([128, 2 * C], f32)
        FCNTv = FCNT[:].rearrange("d (h c) -> d h c", c=C)
        nv.tensor_tensor(out=FCNTv,
                         in0=FSMNT[:].rearrange("d (h p) -> d h p", p=P)[:, :, 0:C],
                         in1=LREP[:, 0:C].unsqueeze(1).broadcast_to([128, 2, C]),
                         op=ALU.mult)
        COSps = psum.tile([C, C], f32, tag="mm", bufs=2)
        for dh in range(2):
            nc.tensor.matmul(COSps[:],
                             lhsT=FSMNT[:, dh * P + C:dh * P + 2 * C],
                             rhs=FCNT[:, dh * C:(dh + 1) * C],
                             start=(dh == 0), stop=(dh == 1))
        COSC = pool.tile([C, C], f32)
        SGN = pool.tile([C, C], f32)
        ts(SGN[:], COSps[:], 0.0, None, ALU.is_lt)
        ts(SGN[:], SGN[:], -2.0, 1.0, ALU.mult, ALU.add)
        nv.tensor_tensor(out=COSC[:], in0=COSps[:], in1=SGN[:], op=ALU.mult)
        ts(COSC[:], COSC[:], 1e-5, 1.0 - 1e-5, ALU.max, ALU.min)
        LGC = pool.tile([C, C], f32)
        ns.activation(LGC[:], COSC[:], AFT.Ln)
        OM = pool.tile([C, C], f32)
        ts(OM[:], COSC[:], -1.0, 1.0, ALU.mult, ALU.add)
        LOM = pool.tile([C, C], f32)
        ns.activation(LOM[:], OM[:], AFT.Ln)
        DIF = pool.tile([C, C], f32)
        nv.tensor_tensor(out=DIF[:], in0=LGC[:], in1=LOM[:], op=ALU.subtract)
        IDM = pool.tile([C, C], f32)
        ts(IDM[:], EYE[:], LAB2[:, 1:2], None, ALU.mult)
        IDS = pool.tile([C, C], f32)
        IDG = pool.tile([C, 1], f32)
        nv.tensor_tensor(out=IDS[:], in0=IDM[:], in1=DIF[:], op=ALU.mult)
        nv.tensor_reduce(out=IDG[:], in_=IDS[:], axis=AX.X, op=ALU.add)
        R1 = pool.tile([C, 1], f32)
        nv.tensor_reduce(out=R1[:], in_=LOM[:], axis=AX.X, op=ALU.add)
        nv.tensor_tensor(out=PR[:, 4:5], in0=IDG[:], in1=R1[:], op=ALU.add)

        COSM = pool.tile([C, C], f32)
        nv.scalar_tensor_tensor(out=COSM[:], in0=EYE[:], scalar=-1e9, in1=COSC[:],
                                op0=ALU.mult, op1=ALU.add)
        OFF = pool.tile([C, 1], f32)
        nv.tensor_reduce(out=OFF[:], in_=COSM[:], axis=AX.X, op=ALU.max)
        QUAL = pool.tile([C, 1], f32)
        ts(QUAL[:], OFF[:], 0.6, None, ALU.is_lt)
        nv.tensor_tensor(out=QUAL[:], in0=QUAL[:], in1=LAB2[:, 1:2], op=ALU.mult)
        nv.tensor_copy(PR[:, 2:3], QUAL[:])
        nv.tensor_tensor(out=PR[:, 3:4], in0=QUAL[:], in1=TERM[:, 1:2], op=ALU.mult)

        # ---- final reduction + scalar assembly ----
        ONES20 = pool.tile([C, 1], f32)
        nv.memset(ONES20[:], 1.0)
        REDps = psum.tile([1, 5], f32, tag="mm", bufs=2)
        nc.tensor.matmul(REDps[:], lhsT=ONES20[:], rhs=PR[:], start=True, stop=True)
        RED = pool.tile([1, 5], f32)
        nv.tensor_copy(RED[:], REDps[:])
        # cols: 0 = S0, 1 = n0, 2 = n1, 3 = S1, 4 = ccf1_sum
        SCR = pool.tile([1, 6], f32)
        ts(SCR[:, 0:1], RED[:, 1:2], 1.0, None, ALU.max)        # max(n0,1)
        nv.reciprocal(SCR[:, 1:2], SCR[:, 0:1])
        nv.tensor_tensor(out=SCR[:, 2:3], in0=RED[:, 0:1], in1=SCR[:, 1:2],
                         op=ALU.mult)                            # l0
        nv.tensor_tensor(out=SCR[:, 2:3], in0=SCR[:, 2:3], in1=RED[:, 3:4],
                         op=ALU.add)                             # l0 + S1
        ts(SCR[:, 3:4], RED[:, 2:3], 1.0, None, ALU.max)        # max(n1,1)
        nv.reciprocal(SCR[:, 4:5], SCR[:, 3:4])
        nv.tensor_tensor(out=SCR[:, 2:3], in0=SCR[:, 2:3], in1=SCR[:, 4:5],
                         op=ALU.mult)                            # loss_cls
        ts(SCR[:, 5:6], RED[:, 1:2], ALPHA, BETA, ALU.mult, ALU.add)  # ccf0
        nv.tensor_tensor(out=SCR[:, 2:3], in0=SCR[:, 2:3], in1=SCR[:, 5:6],
                         op=ALU.add)
        OUT = pool.tile([1, 1], f32)
        ts(OUT[:], RED[:, 4:5], -1.0 / (C * C), None, ALU.mult)
        nv.tensor_tensor(out=OUT[:], in0=OUT[:], in1=SCR[:, 2:3], op=ALU.add)
        nc.sync.dma_start(o_loss, OUT[:])

    nc.compile()
    return nc


# --------------------------------------------------------------------------
# Host marshaling + driver
# --------------------------------------------------------------------------

_CACHE = {}


def _get_programs(hig, low, bg, CP):
    key = (float(hig), float(low), float(bg), CP)
    if key not in _CACHE:
        _CACHE[key] = (_build_a(hig, low, bg, CP), _build_b(CP))
    return _CACHE[key]


def _marshal_a(cam, CP, idxs):
    TOKS = (CP + 3) // 4
    T4 = 4 * TOKS
    grps = []
    t = T4
    while t > 0:
        grps.append(min(8, t))
        t -= grps[-1]
    wct = np.ascontiguousarray(
        W1D.reshape(4, RB, 28).transpose(1, 0, 2).reshape(RB, 4 * 28))
    bbs1 = np.zeros((T4, 16, 1), np.float32)
    for tok in range(T4):
        bbs1[tok] = float((tok % 4) * NPIX)
    selm = np.zeros((TOKS, 16 * T4), np.float32)
    for tok in range(T4):
        selm[tok // 4, 16 * tok:16 * (tok + 1)] = 1.0
    in_maps = []
    for core in range(8):
        b, q = core // NBLK, core % NBLK
        idx = idxs[b]
        camv = np.zeros((CP, NPIX), np.float32)
        if len(idx):
            camv[:len(idx)] = cam[b, idx, q * RB:(q + 1) * RB, :].reshape(
                len(idx), NPIX)
        tsl = idx[TOKS * q:TOKS * q + TOKS]
        # round-1 tokens: (class ct, blk) class-major, each [16, 3136]
        camt = np.zeros((T4, NPIX), np.float32)
        for t2, c in enumerate(tsl):
            camt[4 * t2:4 * (t2 + 1)] = cam[b, c].reshape(4, NPIX)
        m = {
            "camv": camv,
            "selm": selm,
            "idna": np.tile(np.eye(64, dtype=np.float32), (2, 1)),
            "wrt": np.ascontiguousarray(W1D[q * RB:(q + 1) * RB, :]),
            "wct": wct,
        }
        row0 = 0
        for i, g in enumerate(grps):
            m[f"camt{i}"] = np.ascontiguousarray(
                camt[row0:row0 + g]).reshape(16 * g, NPIX // 16)
            m[f"bbs1_{i}"] = np.ascontiguousarray(
                bbs1[row0:row0 + g]).reshape(16 * g, 1)
            row0 += g
        in_maps.append(m)
    return in_maps


def _marshal_b(res_a, fmap, cls_label, proj_weight, CP, idxs):
    P = B * C
    TOKS = (CP + 3) // 4
    # A partials: o_a[core] is [28, CP*28]; scatter slots -> classes,
    # block-major layout [P, NBLK*784]
    ain = np.zeros((P, NBLK * 784), np.float32)
    for core in range(8):
        b, q = core // NBLK, core % NBLK
        a = res_a[core]["o_a"].reshape(28, CP, 28)
        for j, c in enumerate(idxs[b]):
            ain[b * C + c, q * 784:(q + 1) * 784] = np.ascontiguousarray(
                a[:, j, :]).reshape(784)

    cdh = np.zeros((P, NC), np.float32)
    cdw = np.zeros((P, NC), np.float32)
    for core in range(8):
        b, q = core // NBLK, core % NBLK
        ch = res_a[core]["o_ch"].reshape(TOKS, 16, 2)
        cw = res_a[core]["o_cw"].reshape(TOKS, 16, 2)
        tsl = idxs[b][TOKS * q:TOKS * q + TOKS]
        for t, c in enumerate(tsl):
            # sparse_gather compaction order: slot i = f*16 + p
            cdh[b * C + c] = ch[t].T.reshape(NC)
            cdw[b * C + c] = cw[t].T.reshape(NC)

    rnk = np.zeros((P, NC), np.float32)
    rnk[:, :K_TOP] = 1.0 / K_TOP

    fm = np.asarray(fmap, np.float32).reshape(B, D, 7, 112)
    fmi = np.ascontiguousarray(fm.transpose(3, 2, 0, 1)).reshape(112, 7 * B * D)

    labf = np.asarray(cls_label, np.float32)
    return {
        "ain": ain,
        "cdh": cdh,
        "cdw": cdw,
        "rnk": rnk,
        "fmi": fmi,
        "prj": np.ascontiguousarray(
            np.asarray(proj_weight, np.float32).T.reshape(2, 128, C)
            .transpose(1, 0, 2)).reshape(128, 2 * C),
        "lab": labf.reshape(P, 1),
        "lab2": np.ascontiguousarray(labf.T),
        "lrep": np.tile(labf.reshape(1, P), (128, 1)),
        "eye": np.eye(C, dtype=np.float32),
        "i28": np.tile(np.arange(28, dtype=np.float32)[None, :], (128, 1)),
        "mmb": (np.arange(128)[:, None] // NC ==
                np.arange(76)[None, :] - 36).astype(np.float32),
        "idn": np.eye(128, dtype=np.float32),
        "onr": np.ones((1, 128), np.float32),
    }


LAST_EXEC_NS = {}
LAST_RES = {}


def _run(nc, in_maps, core_ids, tag="k"):
    if os.environ.get("BASSK_SIM") == "1":
        from concourse.bass_interp import CoreSim, MultiCoreSim
        if len(core_ids) == 1:
            sim = CoreSim(nc, trace=False, require_finite=False)
            sims = [sim]
        else:
            msim = MultiCoreSim(nc, num_cores=len(core_ids), trace=False,
                                require_finite=False)
            sims = [msim.cores[i] for i in core_ids]
            sim = msim
        for s, m in zip(sims, in_maps):
            for name, arr in m.items():
                s.tensor(name)[:] = arr
        sim.simulate(check_with_hw=False)
        outs = []
        for s in sims:
            d = {}
            for alloc in nc.m.functions[0].allocations:
                if getattr(alloc, "kind", None) == "ExternalOutput":
                    nm = alloc.memorylocations[0].name
                    d[nm] = np.array(s.tensor(nm))
            outs.append(d)
        return outs
    trace = os.environ.get("BASSK_TRACE") == "1"
    if trace:
        try:
            from antenv.axon_hooks import get_axon_ntff_profile_hook  # noqa: F401
        except Exception:
            trace = False
    res = run_bass_kernel_spmd(nc, in_maps, core_ids, trace=trace)
    if res.exec_time_ns is not None:
        LAST_EXEC_NS[tag] = res.exec_time_ns
    LAST_RES[tag] = res
    return res.results


def kernel(fmap, cam, cls_label, proj_weight, feature_contrast,
           hig_thre, low_thre, bg_thre):
    fmap = np.asarray(fmap, np.float32)
    cam = np.asarray(cam, np.float32)
    lab = np.asarray(cls_label, np.float32)
    fc0 = np.asarray(feature_contrast, np.float32)
    if np.any(fc0):
        raise NotImplementedError("kernel specialized to feature_contrast == 0")
    idxs = [np.where(lab[b] > 0.5)[0] for b in range(B)]
    cp_act = max(len(i) for i in idxs)
    CP = min(C, max(4, cp_act))
    nca, ncb = _get_programs(float(hig_thre), float(low_thre), float(bg_thre), CP)

    res_a = _run(nca, _marshal_a(cam, CP, idxs), list(range(8)), tag="A")
    in_b = _marshal_b(res_a, fmap, cls_label, proj_weight, CP, idxs)
    res_b = _run(ncb, [in_b], [0], tag="B")
    loss = np.float32(res_b[0]["o_loss"].reshape(-1)[0])
    return np.asarray(loss, dtype=np.float32).reshape(())


# revision 44
# speedup vs baseline: 1.0260x; 1.0260x over previous
"""Trainium2 Bass kernel for nn_CPCLoss (self-contained).

Strategy (8 NeuronCores, full inputs in / full output out):
  NEFF-A, SPMD on 8 cores — core k = (batch b=k//4, quarter q=k%4).
  Each core:
    * pixel phase on its row-block shard cam[b, :, q*112:(q+1)*112, :]:
      top1 via pairwise tree-max, one-hot argmax via (v >= top1), second
      via tree-max of (v - onehot), keep-gate from thresholds/margin,
      then per-class A-partials A_c = Wr^T @ onehot_c @ Wc through two
      transpose-free PE matmul stages (contract rows first, then cols).
    * exact global per-class top-256 over the FULL image for its 3-class
      slice cam[b, 3q:3q+3, :, :] via the gpsimd topk instruction; the
      top-32 pixel indices are converted to (h, w) on-device.
  Host only reshapes/concats partials (no arithmetic).
  NEFF-B, 1 core — sums the 4 row-block A-partials, computes counts,
  builds the top-25 gather matrix G from the shipped (h, w) candidates
  (static rank weights - no merge needed since candidates are already
  global top-32 sorted), selects coef = count==0 ? G : A/count, computes
  fsm^T = coef @ fmap^T via PE, and runs the 2-step EMA memory-bank scan
  (specialized to the spec's feature_contrast == 0 initial bank) to emit
  the scalar loss.
"""
import math
import os
import sys

os.environ.setdefault("MYCRO_LOCAL_CACHE", "1")
if "/opt/trn_rl_repo" not in sys.path:
    sys.path.insert(0, "/opt/trn_rl_repo")

from contextlib import ExitStack

import numpy as np

from concourse import bacc, bass_isa, mybir, tile
from concourse.bass_utils import run_bass_kernel_spmd

f32 = mybir.dt.float32
f32r = mybir.dt.float32r
bf16 = mybir.dt.bfloat16
i32 = mybir.dt.int32
u32 = mybir.dt.uint32
ALU = mybir.AluOpType
AFT = mybir.ActivationFunctionType
AX = mybir.AxisListType

B, C, D = 2, 20, 256
H = W = 448
FH = FW = 28
K_TOP = 25
NBLK = 4
RB = H // NBLK            # 112
NPIX = RB * W             # 50176 (per row-block shard)
FPIX = H * W              # 200704 (full image)
MARGIN = 0.3
NC = 32                   # candidates kept per (b, class)


def _make_w1d():
    scale = FH / H
    w = np.zeros((H, FH), dtype=np.float64)
    for x in range(H):
        s = (x + 0.5) * scale - 0.5
        i0 = int(np.floor(s))
        f = s - i0
        for i, wt in ((i0, 1.0 - f), (i0 + 1, f)):
            if 0 <= i < FH:
                w[x, i] += wt
        w[x] /= w[x].sum()
    return w.astype(np.float32)


W1D = _make_w1d()


def _emit_topk(nc, out_ap, in_ap, tokens, n):
    g = nc.gpsimd
    return g.add_instruction(bass_isa.InstTopk(
        name=f"I-{nc.next_id()}",
        ins=[g.lower_ap(in_ap, for_isa=True)],
        outs=[g.lower_ap(out_ap, for_isa=True)],
        _tokens=tokens, _n=n, _k=256))


def _tree_max(nc, pool, src, nplane, width, name):
    """Pairwise tree max over `nplane` planes of `width` cols in src
    ([112, nplane*width]); returns a [112, width] tile. Emits ~log2 ops."""
    nv = nc.vector
    cur = src
    cnt = nplane
    lvl = 0
    while cnt > 1:
        half = cnt // 2
        odd = cnt - 2 * half
        dst = pool.tile([RB, (half + odd) * width], f32, name=f"{name}_l{lvl}")
        nv.tensor_tensor(out=dst[:, 0:half * width],
                         in0=cur[:, 0:half * width],
                         in1=cur[:, half * width:2 * half * width], op=ALU.max)
        if odd:
            nv.tensor_copy(dst[:, half * width:(half + 1) * width],
                           cur[:, 2 * half * width:(2 * half + 1) * width])
        cur = dst[:]
        cnt = half + odd
        lvl += 1
    return cur


# --------------------------------------------------------------------------
# NEFF-A
# --------------------------------------------------------------------------

def _build_a(hig, low, bg, CP):
    nc = bacc.Bacc("TRN2", target_bir_lowering=False, debug=False, num_devices=8)
    TOKS = (CP + 3) // 4  # classes handled by this core's global topk
    T4 = 4 * TOKS         # round-1 tokens: (class, row-block)
    grps = []             # round-1 token-tile sizes (<= 8 tokens each)
    t = T4
    while t > 0:
        grps.append(min(8, t))
        t -= grps[-1]

    camv = nc.dram_tensor("camv", [CP, NPIX], f32, kind="ExternalInput").ap()
    camt = [nc.dram_tensor(f"camt{i}", [16 * g, NPIX // 16], f32,
                           kind="ExternalInput").ap() for i, g in enumerate(grps)]
    bbs1 = [nc.dram_tensor(f"bbs1_{i}", [16 * g, 1], f32,
                           kind="ExternalInput").ap() for i, g in enumerate(grps)]
    selm = nc.dram_tensor("selm", [TOKS, 16 * 4 * TOKS], f32,
                          kind="ExternalInput").ap()
    idna = nc.dram_tensor("idna", [128, 64], f32, kind="ExternalInput").ap()
    wrt = nc.dram_tensor("wrt", [RB, 28], f32, kind="ExternalInput").ap()
    wct = nc.dram_tensor("wct", [RB, 4 * 28], f32, kind="ExternalInput").ap()

    o_a = nc.dram_tensor("o_a", [28, CP * 28], f32, kind="ExternalOutput").ap()
    o_ch = nc.dram_tensor("o_ch", [16 * TOKS, 2], f32, kind="ExternalOutput").ap()
    o_cw = nc.dram_tensor("o_cw", [16 * TOKS, 2], f32, kind="ExternalOutput").ap()

    thmax = float(max(hig, low, bg))

    with tile.TileContext(nc) as tc, ExitStack() as ctx:
        pool = ctx.enter_context(tc.tile_pool(name="p", bufs=1))
        psum = ctx.enter_context(tc.tile_pool(name="ps", bufs=1, space="PSUM"))
        nv = nc.vector
        ns = nc.scalar

        # DMA order matters: VP first (it gates the long DVE chain), then the
        # topk inputs, then the small/late tensors.
        VP = pool.tile([RB, CP * W], f32)
        nc.sync.dma_start(VP[:], camv.rearrange("c (r w) -> r c w", w=W))
        VT = [pool.tile([16 * g, NPIX // 16], f32, name=f"vt{i}")
              for i, g in enumerate(grps)]
        for i in range(len(grps)):
            nc.scalar.dma_start(VT[i][:], camt[i])
        WR = pool.tile([RB, 28], f32); nc.gpsimd.dma_start(WR[:], wrt)
        WC = pool.tile([RB, 4 * 28], f32); nc.gpsimd.dma_start(WC[:], wct)
        BBS1 = [pool.tile([16 * g, 1], f32, name=f"bbs1_{i}")
                for i, g in enumerate(grps)]
        for i in range(len(grps)):
            nc.scalar.dma_start(BBS1[i][:], bbs1[i])
        SELM = pool.tile([TOKS, 16 * T4], f32)
        nc.scalar.dma_start(SELM[:], selm)
        IDN64 = pool.tile([128, 64], f32)
        nc.scalar.dma_start(IDN64[:], idna)

        # ---- round 1 topk first in the Pool queue (nothing may block it) ----
        TK1 = [pool.tile([16 * g, 32], u32, name=f"tk1_{i}")
               for i, g in enumerate(grps)]
        for i, g in enumerate(grps):
            _emit_topk(nc, TK1[i][:], VT[i][:], tokens=g, n=NPIX)

        # ---- pixel phase (the long DVE chain goes first in the DVE queue) ----
        T1 = _tree_max(nc, pool, VP[:], CP, W, "t1")

        GE = pool.tile([RB, CP * W], f32)
        GE_cw = GE[:].rearrange("p (c w) -> p c w", w=W)
        V_cw = VP[:].rearrange("p (c w) -> p c w", w=W)
        T1_b = T1.unsqueeze(1).broadcast_to([RB, CP, W])
        nv.tensor_tensor(out=GE_cw, in0=V_cw, in1=T1_b, op=ALU.is_ge)

        # second max: max over c of (v - onehot);  (in-place on VP)
        nv.tensor_tensor(out=VP[:], in0=VP[:], in1=GE[:], op=ALU.subtract)
        SC = _tree_max(nc, pool, VP[:], CP, W, "sc")

        # keep iff top1 >= thmax and margin >= 0.3
        KG = pool.tile([RB, W], f32)
        nv.tensor_scalar(out=KG[:], in0=T1, scalar1=thmax, scalar2=None, op0=ALU.is_ge)
        MG = pool.tile([RB, W], f32)
        nv.tensor_tensor(out=MG[:], in0=T1, in1=SC, op=ALU.subtract)
        nv.tensor_scalar(out=MG[:], in0=MG[:], scalar1=MARGIN, scalar2=None, op0=ALU.is_ge)
        nv.tensor_tensor(out=KG[:], in0=KG[:], in1=MG[:], op=ALU.mult)

        # ---- per-class masks + A partials (two transpose-free stages) ----
        KEEP = pool.tile([RB, CP * W], f32)
        KG_b3 = KG[:].unsqueeze(1)
        Usb = pool.tile([RB, CP * 4 * 28], f32)
        ns1 = 0
        s1ps = []
        s1dst = []

        def flush_s1():
            nonlocal s1ps, s1dst
            for ps_t, (s0, cnt) in zip(s1ps, s1dst):
                ns.copy(Usb[:, s0 * 28:(s0 + cnt) * 28], ps_t[:, 0:cnt * 28])
            s1ps, s1dst = [], []

        cq, rem = divmod(CP, 4)
        chunks = [cq + (1 if g < rem else 0) for g in range(4)]
        c_at = 0
        for g in range(4):
            CQ = chunks[g]
            if CQ == 0:
                continue
            sl = slice(c_at * W, (c_at + CQ) * W)
            nv.tensor_tensor(out=KEEP[:, sl].rearrange("p (c w) -> p c w", w=W),
                             in0=GE[:, sl].rearrange("p (c w) -> p c w", w=W),
                             in1=KG_b3.broadcast_to([RB, CQ, W]), op=ALU.mult)
            for cc in range(CQ):
                c = c_at + cc
                for u in range(4):
                    if ns1 % 5 == 0:
                        s1ps.append(psum.tile([RB, 140], f32, tag="s1", bufs=3,
                                              name=f"s1_{ns1}"))
                        s1dst.append((c * 4 + u, 0))
                    off = (ns1 % 5) * 28
                    nc.tensor.matmul(
                        s1ps[-1][:, off:off + 28],
                        lhsT=KEEP[:, c * W + u * RB:c * W + (u + 1) * RB],
                        rhs=WR[:], start=True, stop=True)
                    s0, cnt = s1dst[-1]
                    s1dst[-1] = (s0, cnt + 1)
                    ns1 += 1
                    if ns1 % 5 == 0:
                        flush_s1()
            c_at += CQ
        flush_s1()

        # stage 2: A_c[i,j] = sum_u U_cu^T @ Wc_u   (5 classes per PSUM bank)
        ngrp = (CP + 4) // 5
        Asb = pool.tile([28, CP * 28], f32)
        for grp in range(ngrp):
            c0 = grp * 5
            c1 = min(c0 + 5, CP)
            aps = psum.tile([28, (c1 - c0) * 28], f32, tag="s2", bufs=2,
                            name=f"s2_{grp}")
            for c in range(c0, c1):
                off = (c - c0) * 28
                for u in range(4):
                    nc.tensor.matmul(
                        aps[:, off:off + 28],
                        lhsT=Usb[:, (c * 4 + u) * 28:(c * 4 + u + 1) * 28],
                        rhs=WC[:, u * 28:(u + 1) * 28],
                        start=(u == 0), stop=(u == 3))
            ns.copy(Asb[:, c0 * 28:c1 * 28], aps[:])
        nc.sync.dma_start(o_a, Asb[:])

        # ---- round 2: global top-25 per class ----
        # Root ops are anchored on KEEP's last column (a real data dep with
        # zero effect) so the greedy in-order scheduler cannot interleave
        # them into the pixel chain: they only become ready after it.
        tc.cur_priority += 100000
        starts = []
        s = 0
        for g in grps:
            starts.append(s)
            s += g
        anc_last = CP * W - 1
        CROWS = pool.tile([TOKS, 128], f32)
        loc = []
        for ct in range(TOKS):
            i = next(j for j, st in enumerate(starts)
                     if st <= 4 * ct < st + grps[j])
            r0 = (4 * ct - starts[i]) * 16
            loc.append((i, r0))
            for k in range(4):
                rr = r0 + 16 * k + 14
                nc.scalar.dma_start(
                    CROWS[ct:ct + 1, 32 * k:32 * k + 32],
                    TK1[i][rr:rr + 2, 0:16].bitcast(f32))
        MV = pool.tile([TOKS, 32], f32)
        CVa = pool.tile([TOKS, 128], f32)
        nv.scalar_tensor_tensor(
            out=CVa[:], in0=KEEP[0:TOKS, anc_last:anc_last + 1]
            .broadcast_to([TOKS, 128]), scalar=0.0, in1=CROWS[:],
            op0=ALU.mult, op1=ALU.add)
        for r in range(4):
            nv.max(out=MV[:, r * 8:(r + 1) * 8], in_=CVa[:])
            nv.match_replace(out=CVa[:], in_to_replace=MV[:, r * 8:(r + 1) * 8],
                             in_values=CVa[:], imm_value=-1.0)
        # sanitize: absent classes (all-zero) must select nothing
        THZ = pool.tile([TOKS, 1], f32)
        nv.tensor_scalar(out=THZ[:], in0=MV[:, 24:25], scalar1=0.0, scalar2=None,
                         op0=ALU.is_le)
        THS = pool.tile([TOKS, 1], f32)
        nv.scalar_tensor_tensor(out=THS[:], in0=THZ[:], scalar=1e30,
                                in1=MV[:, 24:25], op0=ALU.mult, op1=ALU.add)
        # group-level mask + masked-index build (all operands base-0)
        MSKg, MI1g = {}, {}
        for i, g in enumerate(grps):
            THRps = psum.tile([16 * g, 1], f32, tag="thr", bufs=1, name=f"thr{i}")
            nc.tensor.matmul(THRps[:], lhsT=SELM[:, 16 * starts[i]:16 * (starts[i] + g)],
                             rhs=THS[:], start=True, stop=True)
            MSK1 = pool.tile([16 * g, 16], f32, name=f"msk1_{i}")
            nv.tensor_scalar(out=MSK1[:], in0=TK1[i][:, 0:16].bitcast(f32),
                             scalar1=THRps[:], scalar2=None, op0=ALU.is_ge)
            GIC = pool.tile([16 * g, 16], f32, name=f"gic{i}")
            nv.scalar_tensor_tensor(out=GIC[:], in0=MSK1[:], scalar=0.0,
                                    in1=TK1[i][:, 16:32], op0=ALU.mult, op1=ALU.add)
            nv.tensor_scalar(out=GIC[:], in0=GIC[:], scalar1=BBS1[i][:],
                             scalar2=None, op0=ALU.add)
            # masked idx: sel ? gidx : -1   (gidx+1 > 0, so mask*(g+1)-1)
            MI1 = pool.tile([16 * g, 16], f32, name=f"mi1_{i}")
            nv.tensor_scalar(out=MI1[:], in0=GIC[:], scalar1=1.0, scalar2=None,
                             op0=ALU.add)
            nv.tensor_tensor(out=MI1[:], in0=MI1[:], in1=MSK1[:], op=ALU.mult)
            nv.tensor_scalar(out=MI1[:], in0=MI1[:], scalar1=1.0, scalar2=None,
                             op0=ALU.subtract)
            MSKg[i], MI1g[i] = MSK1, MI1
        for ct in range(TOKS):
            i, r0 = loc[ct]
            mps = psum.tile([16, 64], f32, tag="r2ps", bufs=2, name=f"mps{ct}")
            nc.tensor.transpose(mps[:], MI1g[i][r0:r0 + 64, :], IDN64[r0:r0 + 64, :])
            MIc = pool.tile([16, 64], f32, name=f"mic{ct}")
            ns.copy(MIc[:], mps[:])
            CIc = pool.tile([16, 2], f32, name=f"cic{ct}")
            NFc = pool.tile([1, 1], u32, name=f"nfc{ct}")
            nc.gpsimd.sparse_gather(CIc[:], MIc[:], num_found=NFc[:])
            # convert to (h, w); -1 pads land harmlessly (weight 0 in B)
            HI = pool.tile([16, 2], i32, name=f"hi{ct}")
            nv.tensor_scalar(out=HI[:], in0=CIc[:], scalar1=1.0 / 448.0,
                             scalar2=(-0.5 + 1.0 / 1024.0), op0=ALU.mult,
                             op1=ALU.add)
            HF = pool.tile([16, 2], f32, name=f"hf{ct}")
            nv.tensor_copy(HF[:], HI[:])
            WF = pool.tile([16, 2], f32, name=f"wf{ct}")
            nv.scalar_tensor_tensor(out=WF[:], in0=HF[:], scalar=-448.0,
                                    in1=CIc[:], op0=ALU.mult, op1=ALU.add)
            nc.scalar.dma_start(o_ch[16 * ct:16 * (ct + 1), :], HF[:])
            nc.scalar.dma_start(o_cw[16 * ct:16 * (ct + 1), :], WF[:])

    nc.compile()
    return nc


# --------------------------------------------------------------------------
# NEFF-B
# --------------------------------------------------------------------------

def _build_b(CP):
    nc = bacc.Bacc("TRN2", target_bir_lowering=False, debug=False, num_devices=1)
    P = B * C  # 40 (b,c) pairs, b-major

    ain = nc.dram_tensor("ain", [P, NBLK * 784], f32, kind="ExternalInput").ap()
    cdh = nc.dram_tensor("cdh", [P, NC], f32, kind="ExternalInput").ap()
    cdw = nc.dram_tensor("cdw", [P, NC], f32, kind="ExternalInput").ap()
    rnk = nc.dram_tensor("rnk", [P, NC], f32, kind="ExternalInput").ap()
    fmi = nc.dram_tensor("fmi", [112, 7 * B * D], f32, kind="ExternalInput").ap()
    prj = nc.dram_tensor("prj", [128, 2 * C], f32, kind="ExternalInput").ap()
    lab = nc.dram_tensor("lab", [P, 1], f32, kind="ExternalInput").ap()
    lab2 = nc.dram_tensor("lab2", [C, B], f32, kind="ExternalInput").ap()
    lrep = nc.dram_tensor("lrep", [128, P], f32, kind="ExternalInput").ap()
    eye = nc.dram_tensor("eye", [C, C], f32, kind="ExternalInput").ap()
    i28 = nc.dram_tensor("i28", [128, 28], f32, kind="ExternalInput").ap()
    mmb = nc.dram_tensor("mmb", [128, 76], f32, kind="ExternalInput").ap()
    idn = nc.dram_tensor("idn", [128, 128], f32, kind="ExternalInput").ap()
    onr = nc.dram_tensor("onr", [1, 128], f32, kind="ExternalInput").ap()

    o_loss = nc.dram_tensor("o_loss", [1, 1], f32, kind="ExternalOutput").ap()

    LN_EPS = 1e-5
    ALPHA = float(-(math.log(LN_EPS) - math.log1p(-LN_EPS)) / (C * C))
    BETA = float(-math.log1p(-LN_EPS))

    with tile.TileContext(nc) as tc, ExitStack() as ctx:
        pool = ctx.enter_context(tc.tile_pool(name="p", bufs=1))
        psum = ctx.enter_context(tc.tile_pool(name="ps", bufs=1, space="PSUM"))
        nv = nc.vector
        ns = nc.scalar

        CHW = pool.tile([P, 2 * NC], f32)
        nc.sync.dma_start(CHW[:, 0:NC], cdh)
        nc.sync.dma_start(CHW[:, NC:2 * NC], cdw)
        RNK = pool.tile([P, NC], f32); nc.sync.dma_start(RNK[:], rnk)
        AIN = pool.tile([P, NBLK * 784], f32); nc.scalar.dma_start(AIN[:], ain)
        FM = pool.tile([112, 7 * B * D], f32); nc.scalar.dma_start(FM[:], fmi)
        PJT = pool.tile([128, 2 * C], f32); nc.scalar.dma_start(PJT[:], prj)
        LAB = pool.tile([P, 1], f32); nc.scalar.dma_start(LAB[:], lab)
        LAB2 = pool.tile([C, B], f32); nc.scalar.dma_start(LAB2[:], lab2)
        LREP = pool.tile([128, P], f32); nc.gpsimd.dma_start(LREP[:], lrep)
        EYE = pool.tile([C, C], f32); nc.gpsimd.dma_start(EYE[:], eye)
        I28 = pool.tile([128, 28], f32); nc.gpsimd.dma_start(I28[:], i28)
        MMB = pool.tile([128, 76], f32); nc.gpsimd.dma_start(MMB[:], mmb)
        MMBH = pool.tile([128, 76], bf16)
        nv.tensor_copy(MMBH[:], MMB[:])
        IDN = pool.tile([128, 128], f32); nc.gpsimd.dma_start(IDN[:], idn)
        ONR = pool.tile([1, 128], f32); nc.gpsimd.dma_start(ONR[:], onr)

        def ts(dst, src, s1, s2, op0, op1=None):
            nv.tensor_scalar(out=dst, in0=src, scalar1=s1, scalar2=s2, op0=op0,
                             **({"op1": op1} if op1 is not None else {}))

        # ---- interpolation coefficients (written straight into STG) ----
        STG = pool.tile([P, NC * 8], f32)
        STG_v = STG[:].rearrange("p (k a) -> p k a", a=8)

        def sv(idx):
            return STG_v[:, :, idx:idx + 1]

        # combined H|W chain on [P, 2*NC]: u = (x+8.5)/16; fl = floor(u) =
        # rtn((x+0.5)/16); f = u-fl; x0/x1 = clip(fl-1)/clip(fl); w0 = 1-f
        U = pool.tile([P, 2 * NC], f32)
        ts(U[:], CHW[:], 8.5, 1.0 / 16.0, ALU.add, ALU.mult)
        FLI = pool.tile([P, 2 * NC], i32)
        ts(FLI[:], CHW[:], 0.5, 1.0 / 16.0, ALU.add, ALU.mult)
        FLF = pool.tile([P, 2 * NC], f32)
        nv.tensor_copy(FLF[:], FLI[:])
        F = pool.tile([P, 2 * NC], f32)
        nv.tensor_tensor(out=F[:], in0=U[:], in1=FLF[:], op=ALU.subtract)
        X0 = pool.tile([P, 2 * NC], f32)
        ts(X0[:], FLF[:], 1.0, None, ALU.subtract)
        W0 = pool.tile([P, 2 * NC], f32)
        ts(W0[:], F[:], -1.0, 1.0, ALU.mult, ALU.add)
        ts(sv(0), X0[:, 0:NC].unsqueeze(2), 0.0, 27.0, ALU.max, ALU.min)
        ts(sv(1), FLF[:, 0:NC].unsqueeze(2), 0.0, 27.0, ALU.max, ALU.min)
        nv.tensor_copy(sv(2), W0[:, 0:NC].unsqueeze(2))
        nv.tensor_copy(sv(3), F[:, 0:NC].unsqueeze(2))
        ts(sv(4), X0[:, NC:2 * NC].unsqueeze(2), 0.0, 27.0, ALU.max, ALU.min)
        ts(sv(5), FLF[:, NC:2 * NC].unsqueeze(2), 0.0, 27.0, ALU.max, ALU.min)
        nv.tensor_tensor(out=sv(6), in0=W0[:, NC:2 * NC].unsqueeze(2),
                         in1=RNK[:].unsqueeze(2), op=ALU.mult)
        nv.tensor_tensor(out=sv(7), in0=F[:, NC:2 * NC].unsqueeze(2),
                         in1=RNK[:].unsqueeze(2), op=ALU.mult)

        # ---- stage (pair,k) scalars onto partitions: 10 groups of 4 pairs ----
        FLT = pool.tile([128, 80], f32)
        qs = [nc.sync, nc.scalar, nc.gpsimd]
        for g in range(10):
            qs[g % 3].dma_start(
                FLT[:, g * 8:(g + 1) * 8],
                STG[g * 4:(g + 1) * 4, :].rearrange("p (k a) -> p k a", a=8))

        # ---- batched row/col factors + outer products + banded matmuls ----
        I28b = I28[:].unsqueeze(1).broadcast_to([128, 10, 28])

        def fb(idx):
            return FLT[:, idx::8].unsqueeze(2).broadcast_to([128, 10, 28])

        EQ0 = pool.tile([128, 10 * 28], f32)
        EQ0v = EQ0[:].rearrange("p (g a) -> p g a", a=28)
        nv.tensor_tensor(out=EQ0v, in0=I28b, in1=fb(0), op=ALU.is_equal)
        RR = pool.tile([128, 10 * 28], f32)
        RRv = RR[:].rearrange("p (g a) -> p g a", a=28)
        nv.tensor_tensor(out=RRv, in0=EQ0v, in1=fb(2), op=ALU.mult)
        EQ1 = pool.tile([128, 10 * 28], f32)
        EQ1v = EQ1[:].rearrange("p (g a) -> p g a", a=28)
        nv.tensor_tensor(out=EQ1v, in0=I28b, in1=fb(1), op=ALU.is_equal)
        nv.tensor_tensor(out=EQ1v, in0=EQ1v, in1=fb(3), op=ALU.mult)
        nv.tensor_tensor(out=RR[:], in0=RR[:], in1=EQ1[:], op=ALU.add)

        EQ2 = pool.tile([128, 10 * 28], f32)
        EQ2v = EQ2[:].rearrange("p (g a) -> p g a", a=28)
        nv.tensor_tensor(out=EQ2v, in0=I28b, in1=fb(4), op=ALU.is_equal)
        CC = pool.tile([128, 10 * 28], f32)
        CCv = CC[:].rearrange("p (g a) -> p g a", a=28)
        nv.tensor_tensor(out=CCv, in0=EQ2v, in1=fb(6), op=ALU.mult)
        EQ3 = pool.tile([128, 10 * 28], f32)
        EQ3v = EQ3[:].rearrange("p (g a) -> p g a", a=28)
        nv.tensor_tensor(out=EQ3v, in0=I28b, in1=fb(5), op=ALU.is_equal)
        nv.tensor_tensor(out=EQ3v, in0=EQ3v, in1=fb(7), op=ALU.mult)
        nv.tensor_tensor(out=CC[:], in0=CC[:], in1=EQ3[:], op=ALU.add)

        G = pool.tile([P, 784], f32)
        GpsA = psum.tile([P, 392], f32)
        GpsB = psum.tile([P, 392], f32)
        for g in range(10):
            RHS = pool.tile([128, 784], bf16, tag="rhs", bufs=2)
            nv.tensor_tensor(out=RHS[:].rearrange("p (a b) -> p a b", b=28),
                             in0=RR[:, g * 28:(g + 1) * 28].unsqueeze(2)
                             .broadcast_to([128, 28, 28]),
                             in1=CC[:, g * 28:(g + 1) * 28].unsqueeze(1)
                             .broadcast_to([128, 28, 28]),
                             op=ALU.mult)
            lhsT_g = MMBH[:, 36 - 4 * g:76 - 4 * g]
            nc.tensor.matmul(GpsA[:], lhsT=lhsT_g, rhs=RHS[:, 0:392],
                             start=(g == 0), stop=(g == 9))
            nc.tensor.matmul(GpsB[:], lhsT=lhsT_g, rhs=RHS[:, 392:784],
                             start=(g == 0), stop=(g == 9))
        ns.copy(G[:, 0:392], GpsA[:])
        ns.copy(G[:, 392:784], GpsB[:])

        # ---- A sum over 4 row-blocks, counts, coef select ----
        A = pool.tile([P, 784], f32)
        S01 = pool.tile([P, 784], f32)
        nv.tensor_tensor(out=S01[:], in0=AIN[:, 0:784], in1=AIN[:, 784:1568],
                         op=ALU.add)
        nv.tensor_tensor(out=A[:], in0=AIN[:, 1568:2352], in1=AIN[:, 2352:3136],
                         op=ALU.add)
        nv.tensor_tensor(out=A[:], in0=A[:], in1=S01[:], op=ALU.add)
        CNT = pool.tile([P, 1], f32)
        nv.tensor_reduce(out=CNT[:], in_=A[:], axis=AX.X, op=ALU.add)
        ISZ = pool.tile([P, 1], u32)
        ts(ISZ[:], CNT[:], 0.5, None, ALU.is_lt)
        DEN = pool.tile([P, 1], f32)
        ts(DEN[:], CNT[:], 1.0, None, ALU.max)
        RDEN = pool.tile([P, 1], f32)
        nv.reciprocal(RDEN[:], DEN[:])
        AMN = pool.tile([P, 784], f32)
        ts(AMN[:], A[:], RDEN[:], None, ALU.mult)
        COEF = pool.tile([P, 784], f32)
        nv.select(COEF[:], ISZ[:].broadcast_to([P, 784]), G[:], AMN[:])
        ts(COEF[:], COEF[:], LAB[:], None, ALU.mult)

        # ---- coef^T chunks + fsm^T ----
        CT = pool.tile([RB, 7 * P], f32)
        for u in range(7):
            TPS = psum.tile([RB, P], f32, tag="tps", bufs=2)
            nc.tensor.transpose(TPS[:], COEF[:, u * RB:(u + 1) * RB], IDN[:P, :P])
            ns.copy(CT[:, u * P:(u + 1) * P], TPS[:])

        # FSMT[d, (dh, b, c)] chunks: acc over 7 s-chunks
        FSMT = pool.tile([128, 2 * P], f32)
        for dh in range(2):
            for b2 in range(B):
                fps = psum.tile([128, C], f32, tag="fsps", bufs=2)
                for u in range(7):
                    nc.tensor.matmul(
                        fps[:],
                        lhsT=FM[:, u * (B * D) + b2 * D + dh * 128:
                                u * (B * D) + b2 * D + (dh + 1) * 128],
                        rhs=CT[:, u * P + b2 * C:u * P + (b2 + 1) * C],
                        start=(u == 0), stop=(u == 6))
                ns.copy(FSMT[:, dh * P + b2 * C:dh * P + (b2 + 1) * C], fps[:])

        # ---- prescan: fsm norms (both b), softmax/term (both b) ----
        SQ = pool.tile([128, 2 * P], f32)
        nv.tensor_tensor(out=SQ[:], in0=FSMT[:], in1=FSMT[:], op=ALU.mult)
        ONC = pool.tile([128, 1], f32)
        nv.memset(ONC[:], 1.0)
        NNps = psum.tile([1, P], f32, tag="mm", bufs=2)
        for dh in range(2):
            nc.tensor.matmul(NNps[:], lhsT=ONC[:], rhs=SQ[:, dh * P:(dh + 1) * P],
                             start=(dh == 0), stop=(dh == 1))
        NN2 = pool.tile([1, P], f32)
        ts(NN2[:], NNps[:], 1e-24, None, ALU.max)
        NRM = pool.tile([1, P], f32)
        ns.activation(NRM[:], NN2[:], AFT.Sqrt)
        RN = pool.tile([1, P], f32)
        nv.reciprocal(RN[:], NRM[:])
        RNps = psum.tile([128, P], f32, tag="mm", bufs=2)
        nc.tensor.matmul(RNps[:], lhsT=ONR[:], rhs=RN[:], start=True, stop=True)
        FSMNT = pool.tile([128, 2 * P], f32)
        FSMNTv = FSMNT[:].rearrange("d (h p) -> d h p", p=P)
        nv.tensor_tensor(out=FSMNTv, in0=FSMT[:].rearrange("d (h p) -> d h p", p=P),
                         in1=RNps[:].unsqueeze(1).broadcast_to([128, 2, P]),
                         op=ALU.mult)

        # softmax/log terms for both b at once: LOG [C, (b, c')]
        TERM = pool.tile([C, B], f32)
        PR = pool.tile([C, 5], f32)
        for b2 in range(B):
            LOGps = psum.tile([C, C], f32, tag="mm", bufs=2)
            for dh in range(2):
                nc.tensor.matmul(LOGps[:],
                                 lhsT=FSMT[:, dh * P + b2 * C:dh * P + (b2 + 1) * C],
                                 rhs=PJT[:, dh * C:(dh + 1) * C],
                                 start=(dh == 0), stop=(dh == 1))
            MX = pool.tile([C, 1], f32, tag=f"mx{b2}")
            nv.tensor_reduce(out=MX[:], in_=LOGps[:], axis=AX.X, op=ALU.max)
            XT = pool.tile([C, C], f32, tag=f"xt{b2}")
            ts(XT[:], LOGps[:], MX[:], None, ALU.subtract)
            ET = pool.tile([C, C], f32, tag=f"et{b2}")
            ns.activation(ET[:], XT[:], AFT.Exp)
            SM = pool.tile([C, 1], f32, tag=f"sm{b2}")
            nv.tensor_reduce(out=SM[:], in_=ET[:], axis=AX.X, op=ALU.add)
            LGS = pool.tile([C, 1], f32, tag=f"lgs{b2}")
            ns.activation(LGS[:], SM[:], AFT.Ln)
            LGP = pool.tile([C, C], f32, tag=f"lgp{b2}")
            ts(LGP[:], XT[:], LGS[:], -100.0, ALU.subtract, ALU.max)
            SME = pool.tile([C, C], f32, tag=f"sme{b2}")
            nv.tensor_tensor(out=SME[:], in0=SM[:].broadcast_to([C, C]), in1=ET[:],
                             op=ALU.subtract)
            LSME = pool.tile([C, C], f32, tag=f"lsme{b2}")
            ns.activation(LSME[:], SME[:], AFT.Ln)
            L1P = pool.tile([C, C], f32, tag=f"l1p{b2}")
            ts(L1P[:], LSME[:], LGS[:], -100.0, ALU.subtract, ALU.max)
            DD = pool.tile([C, C], f32, tag=f"dd{b2}")
            nv.tensor_tensor(out=DD[:], in0=LGP[:], in1=L1P[:], op=ALU.subtract)
            DDS = pool.tile([C, C], f32, tag=f"dds{b2}")
            DDG = pool.tile([C, 1], f32, tag=f"ddg{b2}")
            nv.tensor_tensor(out=DDS[:], in0=EYE[:], in1=DD[:], op=ALU.mult)
            nv.tensor_reduce(out=DDG[:], in_=DDS[:], axis=AX.X, op=ALU.add)
            RSM = pool.tile([C, 1], f32, tag=f"rsm{b2}")
            nv.tensor_reduce(out=RSM[:], in_=L1P[:], axis=AX.X, op=ALU.add)
            TRM = pool.tile([C, 1], f32, tag=f"trm{b2}")
            nv.tensor_tensor(out=TRM[:], in0=DDG[:], in1=RSM[:], op=ALU.add)
            ts(TERM[:, b2:b2 + 1], TRM[:], -1.0 / C, None, ALU.mult)

        # PR col0 = pres0*term0, col1 = pres0 (iter-0 under fc0 == 0)
        nv.tensor_tensor(out=PR[:, 0:1], in0=TERM[:, 0:1], in1=LAB2[:, 0:1],
                         op=ALU.mult)
        nv.tensor_copy(PR[:, 1:2], LAB2[:, 0:1])

        # ---- iter 1 (fc after iter0 = 0.05 * pres0 * fsm0) ----
        FCNT = pool.tile([128, 2 * C], f32)
        FCNTv = FCNT[:].rearrange("d (h c) -> d h c", c=C)
        nv.tensor_tensor(out=FCNTv,
                         in0=FSMNT[:].rearrange("d (h p) -> d h p", p=P)[:, :, 0:C],
                         in1=LREP[:, 0:C].unsqueeze(1).broadcast_to([128, 2, C]),
                         op=ALU.mult)
        COSps = psum.tile([C, C], f32, tag="mm", bufs=2)
        for dh in range(2):
            nc.tensor.matmul(COSps[:],
                             lhsT=FSMNT[:, dh * P + C:dh * P + 2 * C],
                             rhs=FCNT[:, dh * C:(dh + 1) * C],
                             start=(dh == 0), stop=(dh == 1))
        COSC = pool.tile([C, C], f32)
        SGN = pool.tile([C, C], f32)
        ts(SGN[:], COSps[:], 0.0, None, ALU.is_lt)
        ts(SGN[:], SGN[:], -2.0, 1.0, ALU.mult, ALU.add)
        nv.tensor_tensor(out=COSC[:], in0=COSps[:], in1=SGN[:], op=ALU.mult)
        ts(COSC[:], COSC[:], 1e-5, 1.0 - 1e-5, ALU.max, ALU.min)
        LGC = pool.tile([C, C], f32)
        ns.activation(LGC[:], COSC[:], AFT.Ln)
        OM = pool.tile([C, C], f32)
        ts(OM[:], COSC[:], -1.0, 1.0, ALU.mult, ALU.add)
        LOM = pool.tile([C, C], f32)
        ns.activation(LOM[:], OM[:], AFT.Ln)
        DIF = pool.tile([C, C], f32)
        nv.tensor_tensor(out=DIF[:], in0=LGC[:], in1=LOM[:], op=ALU.subtract)
        IDM = pool.tile([C, C], f32)
        ts(IDM[:], EYE[:], LAB2[:, 1:2], None, ALU.mult)
        IDS = pool.tile([C, C], f32)
        IDG = pool.tile([C, 1], f32)
        nv.tensor_tensor(out=IDS[:], in0=IDM[:], in1=DIF[:], op=ALU.mult)
        nv.tensor_reduce(out=IDG[:], in_=IDS[:], axis=AX.X, op=ALU.add)
        R1 = pool.tile([C, 1], f32)
        nv.tensor_reduce(out=R1[:], in_=LOM[:], axis=AX.X, op=ALU.add)
        nv.tensor_tensor(out=PR[:, 4:5], in0=IDG[:], in1=R1[:], op=ALU.add)

        COSM = pool.tile([C, C], f32)
        nv.scalar_tensor_tensor(out=COSM[:], in0=EYE[:], scalar=-1e9, in1=COSC[:],
                                op0=ALU.mult, op1=ALU.add)
        OFF = pool.tile([C, 1], f32)
        nv.tensor_reduce(out=OFF[:], in_=COSM[:], axis=AX.X, op=ALU.max)
        QUAL = pool.tile([C, 1], f32)
        ts(QUAL[:], OFF[:], 0.6, None, ALU.is_lt)
        nv.tensor_tensor(out=QUAL[:], in0=QUAL[:], in1=LAB2[:, 1:2], op=ALU.mult)
        nv.tensor_copy(PR[:, 2:3], QUAL[:])
        nv.tensor_tensor(out=PR[:, 3:4], in0=QUAL[:], in1=TERM[:, 1:2], op=ALU.mult)

        # ---- final reduction + scalar assembly ----
        ONES20 = pool.tile([C, 1], f32)
        nv.memset(ONES20[:], 1.0)
        REDps = psum.tile([1, 5], f32, tag="mm", bufs=2)
        nc.tensor.matmul(REDps[:], lhsT=ONES20[:], rhs=PR[:], start=True, stop=True)
        RED = pool.tile([1, 5], f32)
        ns.copy(RED[:], REDps[:])
        # cols: 0 = S0, 1 = n0, 2 = n1, 3 = S1, 4 = ccf1_sum
        SCR = pool.tile([1, 6], f32)
        ts(SCR[:, 0:1], RED[:, 1:2], 1.0, None, ALU.max)        # max(n0,1)
        nv.reciprocal(SCR[:, 1:2], SCR[:, 0:1])
        nv.tensor_tensor(out=SCR[:, 2:3], in0=RED[:, 0:1], in1=SCR[:, 1:2],
                         op=ALU.mult)                            # l0
        nv.tensor_tensor(out=SCR[:, 2:3], in0=SCR[:, 2:3], in1=RED[:, 3:4],
                         op=ALU.add)                             # l0 + S1
        ts(SCR[:, 3:4], RED[:, 2:3], 1.0, None, ALU.max)        # max(n1,1)
        nv.reciprocal(SCR[:, 4:5], SCR[:, 3:4])
        nv.tensor_tensor(out=SCR[:, 2:3], in0=SCR[:, 2:3], in1=SCR[:, 4:5],
                         op=ALU.mult)                            # loss_cls
        ts(SCR[:, 5:6], RED[:, 1:2], ALPHA, BETA, ALU.mult, ALU.add)  # ccf0
        nv.tensor_tensor(out=SCR[:, 2:3], in0=SCR[:, 2:3], in1=SCR[:, 5:6],
                         op=ALU.add)
        OUT = pool.tile([1, 1], f32)
        ts(OUT[:], RED[:, 4:5], -1.0 / (C * C), None, ALU.mult)
        nv.tensor_tensor(out=OUT[:], in0=OUT[:], in1=SCR[:, 2:3], op=ALU.add)
        nc.sync.dma_start(o_loss, OUT[:])

    nc.compile()
    return nc


# --------------------------------------------------------------------------
# Host marshaling + driver
# --------------------------------------------------------------------------

_CACHE = {}


def _get_programs(hig, low, bg, CP):
    key = (float(hig), float(low), float(bg), CP)
    if key not in _CACHE:
        _CACHE[key] = (_build_a(hig, low, bg, CP), _build_b(CP))
    return _CACHE[key]


def _marshal_a(cam, CP, idxs):
    TOKS = (CP + 3) // 4
    T4 = 4 * TOKS
    grps = []
    t = T4
    while t > 0:
        grps.append(min(8, t))
        t -= grps[-1]
    wct = np.ascontiguousarray(
        W1D.reshape(4, RB, 28).transpose(1, 0, 2).reshape(RB, 4 * 28))
    bbs1 = np.zeros((T4, 16, 1), np.float32)
    for tok in range(T4):
        bbs1[tok] = float((tok % 4) * NPIX)
    selm = np.zeros((TOKS, 16 * T4), np.float32)
    for tok in range(T4):
        selm[tok // 4, 16 * tok:16 * (tok + 1)] = 1.0
    in_maps = []
    for core in range(8):
        b, q = core // NBLK, core % NBLK
        idx = idxs[b]
        camv = np.zeros((CP, NPIX), np.float32)
        if len(idx):
            camv[:len(idx)] = cam[b, idx, q * RB:(q + 1) * RB, :].reshape(
                len(idx), NPIX)
        tsl = idx[TOKS * q:TOKS * q + TOKS]
        # round-1 tokens: (class ct, blk) class-major, each [16, 3136]
        camt = np.zeros((T4, NPIX), np.float32)
        for t2, c in enumerate(tsl):
            camt[4 * t2:4 * (t2 + 1)] = cam[b, c].reshape(4, NPIX)
        m = {
            "camv": camv,
            "selm": selm,
            "idna": np.tile(np.eye(64, dtype=np.float32), (2, 1)),
            "wrt": np.ascontiguousarray(W1D[q * RB:(q + 1) * RB, :]),
            "wct": wct,
        }
        row0 = 0
        for i, g in enumerate(grps):
            m[f"camt{i}"] = np.ascontiguousarray(
                camt[row0:row0 + g]).reshape(16 * g, NPIX // 16)
            m[f"bbs1_{i}"] = np.ascontiguousarray(
                bbs1[row0:row0 + g]).reshape(16 * g, 1)
            row0 += g
        in_maps.append(m)
    return in_maps


def _marshal_b(res_a, fmap, cls_label, proj_weight, CP, idxs):
    P = B * C
    TOKS = (CP + 3) // 4
    # A partials: o_a[core] is [28, CP*28]; scatter slots -> classes,
    # block-major layout [P, NBLK*784]
    ain = np.zeros((P, NBLK * 784), np.float32)
    for core in range(8):
        b, q = core // NBLK, core % NBLK
        a = res_a[core]["o_a"].reshape(28, CP, 28)
        for j, c in enumerate(idxs[b]):
            ain[b * C + c, q * 784:(q + 1) * 784] = np.ascontiguousarray(
                a[:, j, :]).reshape(784)

    cdh = np.zeros((P, NC), np.float32)
    cdw = np.zeros((P, NC), np.float32)
    for core in range(8):
        b, q = core // NBLK, core % NBLK
        ch = res_a[core]["o_ch"].reshape(TOKS, 16, 2)
        cw = res_a[core]["o_cw"].reshape(TOKS, 16, 2)
        tsl = idxs[b][TOKS * q:TOKS * q + TOKS]
        for t, c in enumerate(tsl):
            # sparse_gather compaction order: slot i = f*16 + p
            cdh[b * C + c] = ch[t].T.reshape(NC)
            cdw[b * C + c] = cw[t].T.reshape(NC)

    rnk = np.zeros((P, NC), np.float32)
    rnk[:, :K_TOP] = 1.0 / K_TOP

    fm = np.asarray(fmap, np.float32).reshape(B, D, 7, 112)
    fmi = np.ascontiguousarray(fm.transpose(3, 2, 0, 1)).reshape(112, 7 * B * D)

    labf = np.asarray(cls_label, np.float32)
    return {
        "ain": ain,
        "cdh": cdh,
        "cdw": cdw,
        "rnk": rnk,
        "fmi": fmi,
        "prj": np.ascontiguousarray(
            np.asarray(proj_weight, np.float32).T.reshape(2, 128, C)
            .transpose(1, 0, 2)).reshape(128, 2 * C),
        "lab": labf.reshape(P, 1),
        "lab2": np.ascontiguousarray(labf.T),
        "lrep": np.tile(labf.reshape(1, P), (128, 1)),
        "eye": np.eye(C, dtype=np.float32),
        "i28": np.tile(np.arange(28, dtype=np.float32)[None, :], (128, 1)),
        "mmb": (np.arange(128)[:, None] // NC ==
                np.arange(76)[None, :] - 36).astype(np.float32),
        "idn": np.eye(128, dtype=np.float32),
        "onr": np.ones((1, 128), np.float32),
    }


LAST_EXEC_NS = {}
LAST_RES = {}


def _run(nc, in_maps, core_ids, tag="k"):
    if os.environ.get("BASSK_SIM") == "1":
        from concourse.bass_interp import CoreSim, MultiCoreSim
        if len(core_ids) == 1:
            sim = CoreSim(nc, trace=False, require_finite=False)
            sims = [sim]
        else:
            msim = MultiCoreSim(nc, num_cores=len(core_ids), trace=False,
                                require_finite=False)
            sims = [msim.cores[i] for i in core_ids]
            sim = msim
        for s, m in zip(sims, in_maps):
            for name, arr in m.items():
                s.tensor(name)[:] = arr
        sim.simulate(check_with_hw=False)
        outs = []
        for s in sims:
            d = {}
            for alloc in nc.m.functions[0].allocations:
                if getattr(alloc, "kind", None) == "ExternalOutput":
                    nm = alloc.memorylocations[0].name
                    d[nm] = np.array(s.tensor(nm))
            outs.append(d)
        return outs
    trace = os.environ.get("BASSK_TRACE") == "1"
    if trace:
        try:
            from antenv.axon_hooks import get_axon_ntff_profile_hook  # noqa: F401
        except Exception:
            trace = False
    res = run_bass_kernel_spmd(nc, in_maps, core_ids, trace=trace)
    if res.exec_time_ns is not None:
        LAST_EXEC_NS[tag] = res.exec_time_ns
    LAST_RES[tag] = res
    return res.results


def kernel(fmap, cam, cls_label, proj_weight, feature_contrast,
           hig_thre, low_thre, bg_thre):
    fmap = np.asarray(fmap, np.float32)
    cam = np.asarray(cam, np.float32)
    lab = np.asarray(cls_label, np.float32)
    fc0 = np.asarray(feature_contrast, np.float32)
    if np.any(fc0):
        raise NotImplementedError("kernel specialized to feature_contrast == 0")
    idxs = [np.where(lab[b] > 0.5)[0] for b in range(B)]
    cp_act = max(len(i) for i in idxs)
    CP = min(C, max(4, cp_act))
    nca, ncb = _get_programs(float(hig_thre), float(low_thre), float(bg_thre), CP)

    res_a = _run(nca, _marshal_a(cam, CP, idxs), list(range(8)), tag="A")
    in_b = _marshal_b(res_a, fmap, cls_label, proj_weight, CP, idxs)
    res_b = _run(ncb, [in_b], [0], tag="B")
    loss = np.float32(res_b[0]["o_loss"].reshape(-1)[0])
    return np.asarray(loss, dtype=np.float32).reshape(())


# revision 45
# speedup vs baseline: 1.0273x; 1.0012x over previous
"""Trainium2 Bass kernel for nn_CPCLoss (self-contained).

Strategy (8 NeuronCores, full inputs in / full output out):
  NEFF-A, SPMD on 8 cores — core k = (batch b=k//4, quarter q=k%4).
  Each core:
    * pixel phase on its row-block shard cam[b, :, q*112:(q+1)*112, :]:
      top1 via pairwise tree-max, one-hot argmax via (v >= top1), second
      via tree-max of (v - onehot), keep-gate from thresholds/margin,
      then per-class A-partials A_c = Wr^T @ onehot_c @ Wc through two
      transpose-free PE matmul stages (contract rows first, then cols).
    * exact global per-class top-256 over the FULL image for its 3-class
      slice cam[b, 3q:3q+3, :, :] via the gpsimd topk instruction; the
      top-32 pixel indices are converted to (h, w) on-device.
  Host only reshapes/concats partials (no arithmetic).
  NEFF-B, 1 core — sums the 4 row-block A-partials, computes counts,
  builds the top-25 gather matrix G from the shipped (h, w) candidates
  (static rank weights - no merge needed since candidates are already
  global top-32 sorted), selects coef = count==0 ? G : A/count, computes
  fsm^T = coef @ fmap^T via PE, and runs the 2-step EMA memory-bank scan
  (specialized to the spec's feature_contrast == 0 initial bank) to emit
  the scalar loss.
"""
import math
import os
import sys

os.environ.setdefault("MYCRO_LOCAL_CACHE", "1")
if "/opt/trn_rl_repo" not in sys.path:
    sys.path.insert(0, "/opt/trn_rl_repo")

from contextlib import ExitStack

import numpy as np

from concourse import bacc, bass_isa, mybir, tile
from concourse.bass_utils import run_bass_kernel_spmd

f32 = mybir.dt.float32
f32r = mybir.dt.float32r
bf16 = mybir.dt.bfloat16
i32 = mybir.dt.int32
u32 = mybir.dt.uint32
ALU = mybir.AluOpType
AFT = mybir.ActivationFunctionType
AX = mybir.AxisListType

B, C, D = 2, 20, 256
H = W = 448
FH = FW = 28
K_TOP = 25
NBLK = 4
RB = H // NBLK            # 112
NPIX = RB * W             # 50176 (per row-block shard)
FPIX = H * W              # 200704 (full image)
MARGIN = 0.3
NC = 32                   # candidates kept per (b, class)


def _make_w1d():
    scale = FH / H
    w = np.zeros((H, FH), dtype=np.float64)
    for x in range(H):
        s = (x + 0.5) * scale - 0.5
        i0 = int(np.floor(s))
        f = s - i0
        for i, wt in ((i0, 1.0 - f), (i0 + 1, f)):
            if 0 <= i < FH:
                w[x, i] += wt
        w[x] /= w[x].sum()
    return w.astype(np.float32)


W1D = _make_w1d()


def _emit_topk(nc, out_ap, in_ap, tokens, n):
    g = nc.gpsimd
    return g.add_instruction(bass_isa.InstTopk(
        name=f"I-{nc.next_id()}",
        ins=[g.lower_ap(in_ap, for_isa=True)],
        outs=[g.lower_ap(out_ap, for_isa=True)],
        _tokens=tokens, _n=n, _k=256))


def _tree_max(nc, pool, src, nplane, width, name):
    """Pairwise tree max over `nplane` planes of `width` cols in src
    ([112, nplane*width]); returns a [112, width] tile. Emits ~log2 ops."""
    nv = nc.vector
    cur = src
    cnt = nplane
    lvl = 0
    while cnt > 1:
        half = cnt // 2
        odd = cnt - 2 * half
        dst = pool.tile([RB, (half + odd) * width], f32, name=f"{name}_l{lvl}")
        nv.tensor_tensor(out=dst[:, 0:half * width],
                         in0=cur[:, 0:half * width],
                         in1=cur[:, half * width:2 * half * width], op=ALU.max)
        if odd:
            nv.tensor_copy(dst[:, half * width:(half + 1) * width],
                           cur[:, 2 * half * width:(2 * half + 1) * width])
        cur = dst[:]
        cnt = half + odd
        lvl += 1
    return cur


# --------------------------------------------------------------------------
# NEFF-A
# --------------------------------------------------------------------------

def _build_a(hig, low, bg, CP):
    nc = bacc.Bacc("TRN2", target_bir_lowering=False, debug=False, num_devices=8)
    TOKS = (CP + 3) // 4  # classes handled by this core's global topk
    T4 = 4 * TOKS         # round-1 tokens: (class, row-block)
    grps = []             # round-1 token-tile sizes (<= 8 tokens each)
    t = T4
    while t > 0:
        grps.append(min(8, t))
        t -= grps[-1]

    camv = nc.dram_tensor("camv", [CP, NPIX], f32, kind="ExternalInput").ap()
    camt = [nc.dram_tensor(f"camt{i}", [16 * g, NPIX // 16], f32,
                           kind="ExternalInput").ap() for i, g in enumerate(grps)]
    bbs1 = [nc.dram_tensor(f"bbs1_{i}", [16 * g, 1], f32,
                           kind="ExternalInput").ap() for i, g in enumerate(grps)]
    selm = nc.dram_tensor("selm", [TOKS, 16 * 4 * TOKS], f32,
                          kind="ExternalInput").ap()
    idna = nc.dram_tensor("idna", [128, 64], f32, kind="ExternalInput").ap()
    wrt = nc.dram_tensor("wrt", [RB, 28], f32, kind="ExternalInput").ap()
    wct = nc.dram_tensor("wct", [RB, 4 * 28], f32, kind="ExternalInput").ap()

    o_a = nc.dram_tensor("o_a", [28, CP * 28], f32, kind="ExternalOutput").ap()
    o_ch = nc.dram_tensor("o_ch", [16 * TOKS, 2], f32, kind="ExternalOutput").ap()
    o_cw = nc.dram_tensor("o_cw", [16 * TOKS, 2], f32, kind="ExternalOutput").ap()

    thmax = float(max(hig, low, bg))

    with tile.TileContext(nc) as tc, ExitStack() as ctx:
        pool = ctx.enter_context(tc.tile_pool(name="p", bufs=1))
        psum = ctx.enter_context(tc.tile_pool(name="ps", bufs=1, space="PSUM"))
        nv = nc.vector
        ns = nc.scalar

        # DMA order matters: VP first (it gates the long DVE chain), then the
        # topk inputs, then the small/late tensors.
        VP = pool.tile([RB, CP * W], f32)
        nc.sync.dma_start(VP[:], camv.rearrange("c (r w) -> r c w", w=W))
        VT = [pool.tile([16 * g, NPIX // 16], f32, name=f"vt{i}")
              for i, g in enumerate(grps)]
        for i in range(len(grps)):
            nc.scalar.dma_start(VT[i][:], camt[i])
        WR = pool.tile([RB, 28], f32); nc.gpsimd.dma_start(WR[:], wrt)
        WC = pool.tile([RB, 4 * 28], f32); nc.gpsimd.dma_start(WC[:], wct)
        BBS1 = [pool.tile([16 * g, 1], f32, name=f"bbs1_{i}")
                for i, g in enumerate(grps)]
        for i in range(len(grps)):
            nc.scalar.dma_start(BBS1[i][:], bbs1[i])
        SELM = pool.tile([TOKS, 16 * T4], f32)
        nc.scalar.dma_start(SELM[:], selm)
        IDN64 = pool.tile([128, 64], f32)
        nc.scalar.dma_start(IDN64[:], idna)

        # ---- round 1 topk first in the Pool queue (nothing may block it) ----
        TK1 = [pool.tile([16 * g, 32], u32, name=f"tk1_{i}")
               for i, g in enumerate(grps)]
        for i, g in enumerate(grps):
            _emit_topk(nc, TK1[i][:], VT[i][:], tokens=g, n=NPIX)

        # ---- pixel phase (the long DVE chain goes first in the DVE queue) ----
        T1 = _tree_max(nc, pool, VP[:], CP, W, "t1")

        GE = pool.tile([RB, CP * W], f32)
        GE_cw = GE[:].rearrange("p (c w) -> p c w", w=W)
        V_cw = VP[:].rearrange("p (c w) -> p c w", w=W)
        T1_b = T1.unsqueeze(1).broadcast_to([RB, CP, W])
        nv.tensor_tensor(out=GE_cw, in0=V_cw, in1=T1_b, op=ALU.is_ge)

        # second max: max over c of (v - onehot);  (in-place on VP)
        nv.tensor_tensor(out=VP[:], in0=VP[:], in1=GE[:], op=ALU.subtract)
        SC = _tree_max(nc, pool, VP[:], CP, W, "sc")

        # keep iff top1 >= thmax and margin >= 0.3
        KG = pool.tile([RB, W], f32)
        nv.tensor_scalar(out=KG[:], in0=T1, scalar1=thmax, scalar2=None, op0=ALU.is_ge)
        MG = pool.tile([RB, W], f32)
        nv.tensor_tensor(out=MG[:], in0=T1, in1=SC, op=ALU.subtract)
        nv.tensor_scalar(out=MG[:], in0=MG[:], scalar1=MARGIN, scalar2=None, op0=ALU.is_ge)
        nv.tensor_tensor(out=KG[:], in0=KG[:], in1=MG[:], op=ALU.mult)

        # ---- per-class masks + A partials (two transpose-free stages) ----
        KEEP = pool.tile([RB, CP * W], f32)
        KG_b3 = KG[:].unsqueeze(1)
        Usb = pool.tile([RB, CP * 4 * 28], f32)
        ns1 = 0
        s1ps = []
        s1dst = []

        def flush_s1():
            nonlocal s1ps, s1dst
            for ps_t, (s0, cnt) in zip(s1ps, s1dst):
                ns.copy(Usb[:, s0 * 28:(s0 + cnt) * 28], ps_t[:, 0:cnt * 28])
            s1ps, s1dst = [], []

        cq, rem = divmod(CP, 4)
        chunks = [cq + (1 if g < rem else 0) for g in range(4)]
        c_at = 0
        for g in range(4):
            CQ = chunks[g]
            if CQ == 0:
                continue
            sl = slice(c_at * W, (c_at + CQ) * W)
            nv.tensor_tensor(out=KEEP[:, sl].rearrange("p (c w) -> p c w", w=W),
                             in0=GE[:, sl].rearrange("p (c w) -> p c w", w=W),
                             in1=KG_b3.broadcast_to([RB, CQ, W]), op=ALU.mult)
            for cc in range(CQ):
                c = c_at + cc
                for u in range(4):
                    if ns1 % 5 == 0:
                        s1ps.append(psum.tile([RB, 140], f32, tag="s1", bufs=3,
                                              name=f"s1_{ns1}"))
                        s1dst.append((c * 4 + u, 0))
                    off = (ns1 % 5) * 28
                    nc.tensor.matmul(
                        s1ps[-1][:, off:off + 28],
                        lhsT=KEEP[:, c * W + u * RB:c * W + (u + 1) * RB],
                        rhs=WR[:], start=True, stop=True)
                    s0, cnt = s1dst[-1]
                    s1dst[-1] = (s0, cnt + 1)
                    ns1 += 1
                    if ns1 % 5 == 0:
                        flush_s1()
            c_at += CQ
        flush_s1()

        # stage 2: A_c[i,j] = sum_u U_cu^T @ Wc_u   (5 classes per PSUM bank)
        ngrp = (CP + 4) // 5
        Asb = pool.tile([28, CP * 28], f32)
        for grp in range(ngrp):
            c0 = grp * 5
            c1 = min(c0 + 5, CP)
            aps = psum.tile([28, (c1 - c0) * 28], f32, tag="s2", bufs=2,
                            name=f"s2_{grp}")
            for c in range(c0, c1):
                off = (c - c0) * 28
                for u in range(4):
                    nc.tensor.matmul(
                        aps[:, off:off + 28],
                        lhsT=Usb[:, (c * 4 + u) * 28:(c * 4 + u + 1) * 28],
                        rhs=WC[:, u * 28:(u + 1) * 28],
                        start=(u == 0), stop=(u == 3))
            ns.copy(Asb[:, c0 * 28:c1 * 28], aps[:])
        nc.sync.dma_start(o_a, Asb[:])

        # ---- round 2: global top-25 per class ----
        # Root ops are anchored on KEEP's last column (a real data dep with
        # zero effect) so the greedy in-order scheduler cannot interleave
        # them into the pixel chain: they only become ready after it.
        tc.cur_priority += 100000
        starts = []
        s = 0
        for g in grps:
            starts.append(s)
            s += g
        anc_last = CP * W - 1
        CROWS = pool.tile([TOKS, 128], f32)
        loc = []
        for ct in range(TOKS):
            i = next(j for j, st in enumerate(starts)
                     if st <= 4 * ct < st + grps[j])
            r0 = (4 * ct - starts[i]) * 16
            loc.append((i, r0))
            for k in range(4):
                rr = r0 + 16 * k + 14
                nc.scalar.dma_start(
                    CROWS[ct:ct + 1, 32 * k:32 * k + 32],
                    TK1[i][rr:rr + 2, 0:16].bitcast(f32))
        MV = pool.tile([TOKS, 32], f32)
        CVa = pool.tile([TOKS, 128], f32)
        nv.scalar_tensor_tensor(
            out=CVa[:], in0=KEEP[0:TOKS, anc_last:anc_last + 1]
            .broadcast_to([TOKS, 128]), scalar=0.0, in1=CROWS[:],
            op0=ALU.mult, op1=ALU.add)
        for r in range(4):
            nv.max(out=MV[:, r * 8:(r + 1) * 8], in_=CVa[:])
            nv.match_replace(out=CVa[:], in_to_replace=MV[:, r * 8:(r + 1) * 8],
                             in_values=CVa[:], imm_value=-1.0)
        # sanitize: absent classes (all-zero) must select nothing
        THZ = pool.tile([TOKS, 1], f32)
        nv.tensor_scalar(out=THZ[:], in0=MV[:, 24:25], scalar1=0.0, scalar2=None,
                         op0=ALU.is_le)
        THS = pool.tile([TOKS, 1], f32)
        nv.scalar_tensor_tensor(out=THS[:], in0=THZ[:], scalar=1e30,
                                in1=MV[:, 24:25], op0=ALU.mult, op1=ALU.add)
        # group-level mask + masked-index build (all operands base-0)
        MSKg, MI1g = {}, {}
        for i, g in enumerate(grps):
            THRps = psum.tile([16 * g, 1], f32, tag="thr", bufs=1, name=f"thr{i}")
            nc.tensor.matmul(THRps[:], lhsT=SELM[:, 16 * starts[i]:16 * (starts[i] + g)],
                             rhs=THS[:], start=True, stop=True)
            MSK1 = pool.tile([16 * g, 16], f32, name=f"msk1_{i}")
            nv.tensor_scalar(out=MSK1[:], in0=TK1[i][:, 0:16].bitcast(f32),
                             scalar1=THRps[:], scalar2=None, op0=ALU.is_ge)
            GIC = pool.tile([16 * g, 16], f32, name=f"gic{i}")
            nv.scalar_tensor_tensor(out=GIC[:], in0=MSK1[:], scalar=0.0,
                                    in1=TK1[i][:, 16:32], op0=ALU.mult, op1=ALU.add)
            nv.tensor_scalar(out=GIC[:], in0=GIC[:], scalar1=BBS1[i][:],
                             scalar2=None, op0=ALU.add)
            # masked idx: sel ? gidx : -1   (gidx+1 > 0, so mask*(g+1)-1)
            MI1 = pool.tile([16 * g, 16], f32, name=f"mi1_{i}")
            nv.tensor_scalar(out=MI1[:], in0=GIC[:], scalar1=1.0, scalar2=None,
                             op0=ALU.add)
            nv.tensor_tensor(out=MI1[:], in0=MI1[:], in1=MSK1[:], op=ALU.mult)
            nv.tensor_scalar(out=MI1[:], in0=MI1[:], scalar1=1.0, scalar2=None,
                             op0=ALU.subtract)
            MSKg[i], MI1g[i] = MSK1, MI1
        for ct in range(TOKS):
            i, r0 = loc[ct]
            mps = psum.tile([16, 64], f32, tag="r2ps", bufs=2, name=f"mps{ct}")
            nc.tensor.transpose(mps[:], MI1g[i][r0:r0 + 64, :], IDN64[r0:r0 + 64, :])
            MIc = pool.tile([16, 64], f32, name=f"mic{ct}")
            ns.copy(MIc[:], mps[:])
            CIc = pool.tile([16, 2], f32, name=f"cic{ct}")
            NFc = pool.tile([1, 1], u32, name=f"nfc{ct}")
            nc.gpsimd.sparse_gather(CIc[:], MIc[:], num_found=NFc[:])
            # convert to (h, w); -1 pads land harmlessly (weight 0 in B)
            HI = pool.tile([16, 2], i32, name=f"hi{ct}")
            nv.tensor_scalar(out=HI[:], in0=CIc[:], scalar1=1.0 / 448.0,
                             scalar2=(-0.5 + 1.0 / 1024.0), op0=ALU.mult,
                             op1=ALU.add)
            HF = pool.tile([16, 2], f32, name=f"hf{ct}")
            nv.tensor_copy(HF[:], HI[:])
            WF = pool.tile([16, 2], f32, name=f"wf{ct}")
            nv.scalar_tensor_tensor(out=WF[:], in0=HF[:], scalar=-448.0,
                                    in1=CIc[:], op0=ALU.mult, op1=ALU.add)
            nc.scalar.dma_start(o_ch[16 * ct:16 * (ct + 1), :], HF[:])
            nc.scalar.dma_start(o_cw[16 * ct:16 * (ct + 1), :], WF[:])

    nc.compile()
    return nc


# --------------------------------------------------------------------------
# NEFF-B
# --------------------------------------------------------------------------

def _build_b(CP):
    nc = bacc.Bacc("TRN2", target_bir_lowering=False, debug=False, num_devices=1)
    P = B * C  # 40 (b,c) pairs, b-major

    ain = nc.dram_tensor("ain", [P, NBLK * 784], f32, kind="ExternalInput").ap()
    cdh = nc.dram_tensor("cdh", [P, NC], f32, kind="ExternalInput").ap()
    cdw = nc.dram_tensor("cdw", [P, NC], f32, kind="ExternalInput").ap()
    rnk = nc.dram_tensor("rnk", [P, NC], f32, kind="ExternalInput").ap()
    fmi = nc.dram_tensor("fmi", [112, 7 * B * D], f32, kind="ExternalInput").ap()
    prj = nc.dram_tensor("prj", [128, 2 * C], f32, kind="ExternalInput").ap()
    lab = nc.dram_tensor("lab", [P, 1], f32, kind="ExternalInput").ap()
    lab2 = nc.dram_tensor("lab2", [C, B], f32, kind="ExternalInput").ap()
    lrep = nc.dram_tensor("lrep", [128, P], f32, kind="ExternalInput").ap()
    eye = nc.dram_tensor("eye", [C, C], f32, kind="ExternalInput").ap()
    i28 = nc.dram_tensor("i28", [128, 28], f32, kind="ExternalInput").ap()
    mmb = nc.dram_tensor("mmb", [128, 76], f32, kind="ExternalInput").ap()
    idn = nc.dram_tensor("idn", [128, 128], f32, kind="ExternalInput").ap()
    onr = nc.dram_tensor("onr", [1, 128], f32, kind="ExternalInput").ap()

    o_loss = nc.dram_tensor("o_loss", [1, 1], f32, kind="ExternalOutput").ap()

    LN_EPS = 1e-5
    ALPHA = float(-(math.log(LN_EPS) - math.log1p(-LN_EPS)) / (C * C))
    BETA = float(-math.log1p(-LN_EPS))

    with tile.TileContext(nc) as tc, ExitStack() as ctx:
        pool = ctx.enter_context(tc.tile_pool(name="p", bufs=1))
        psum = ctx.enter_context(tc.tile_pool(name="ps", bufs=1, space="PSUM"))
        nv = nc.vector
        ns = nc.scalar

        CHW = pool.tile([P, 2 * NC], f32)
        nc.sync.dma_start(CHW[:, 0:NC], cdh)
        nc.sync.dma_start(CHW[:, NC:2 * NC], cdw)
        RNK = pool.tile([P, NC], f32); nc.sync.dma_start(RNK[:], rnk)
        AIN = pool.tile([P, NBLK * 784], f32); nc.scalar.dma_start(AIN[:], ain)
        FM = pool.tile([112, 7 * B * D], f32); nc.scalar.dma_start(FM[:], fmi)
        PJT = pool.tile([128, 2 * C], f32); nc.scalar.dma_start(PJT[:], prj)
        LAB = pool.tile([P, 1], f32); nc.scalar.dma_start(LAB[:], lab)
        LAB2 = pool.tile([C, B], f32); nc.scalar.dma_start(LAB2[:], lab2)
        LREP = pool.tile([128, P], f32); nc.gpsimd.dma_start(LREP[:], lrep)
        EYE = pool.tile([C, C], f32); nc.gpsimd.dma_start(EYE[:], eye)
        I28 = pool.tile([128, 28], f32); nc.gpsimd.dma_start(I28[:], i28)
        MMB = pool.tile([128, 76], f32); nc.gpsimd.dma_start(MMB[:], mmb)
        MMBH = pool.tile([128, 76], bf16)
        nv.tensor_copy(MMBH[:], MMB[:])
        IDN = pool.tile([128, 128], f32); nc.gpsimd.dma_start(IDN[:], idn)
        ONR = pool.tile([1, 128], f32); nc.gpsimd.dma_start(ONR[:], onr)

        def ts(dst, src, s1, s2, op0, op1=None):
            nv.tensor_scalar(out=dst, in0=src, scalar1=s1, scalar2=s2, op0=op0,
                             **({"op1": op1} if op1 is not None else {}))

        # ---- interpolation coefficients (written straight into STG) ----
        STG = pool.tile([P, NC * 8], f32)
        STG_v = STG[:].rearrange("p (k a) -> p k a", a=8)

        def sv(idx):
            return STG_v[:, :, idx:idx + 1]

        # combined H|W chain on [P, 2*NC]: u = (x+8.5)/16; fl = floor(u) =
        # rtn((x+0.5)/16); f = u-fl; x0/x1 = clip(fl-1)/clip(fl); w0 = 1-f
        U = pool.tile([P, 2 * NC], f32)
        ts(U[:], CHW[:], 8.5, 1.0 / 16.0, ALU.add, ALU.mult)
        FLI = pool.tile([P, 2 * NC], i32)
        ts(FLI[:], CHW[:], 0.5, 1.0 / 16.0, ALU.add, ALU.mult)
        FLF = pool.tile([P, 2 * NC], f32)
        nv.tensor_copy(FLF[:], FLI[:])
        F = pool.tile([P, 2 * NC], f32)
        nv.tensor_tensor(out=F[:], in0=U[:], in1=FLF[:], op=ALU.subtract)
        X0 = pool.tile([P, 2 * NC], f32)
        ts(X0[:], FLF[:], 1.0, None, ALU.subtract)
        W0 = pool.tile([P, 2 * NC], f32)
        ts(W0[:], F[:], -1.0, 1.0, ALU.mult, ALU.add)
        ts(sv(0), X0[:, 0:NC].unsqueeze(2), 0.0, 27.0, ALU.max, ALU.min)
        ts(sv(1), FLF[:, 0:NC].unsqueeze(2), 0.0, 27.0, ALU.max, ALU.min)
        nv.tensor_copy(sv(2), W0[:, 0:NC].unsqueeze(2))
        nv.tensor_copy(sv(3), F[:, 0:NC].unsqueeze(2))
        ts(sv(4), X0[:, NC:2 * NC].unsqueeze(2), 0.0, 27.0, ALU.max, ALU.min)
        ts(sv(5), FLF[:, NC:2 * NC].unsqueeze(2), 0.0, 27.0, ALU.max, ALU.min)
        nv.tensor_tensor(out=sv(6), in0=W0[:, NC:2 * NC].unsqueeze(2),
                         in1=RNK[:].unsqueeze(2), op=ALU.mult)
        nv.tensor_tensor(out=sv(7), in0=F[:, NC:2 * NC].unsqueeze(2),
                         in1=RNK[:].unsqueeze(2), op=ALU.mult)

        # ---- stage (pair,k) scalars onto partitions: 10 groups of 4 pairs ----
        FLT = pool.tile([128, 80], f32)
        qs = [nc.sync, nc.scalar, nc.gpsimd]
        for g in range(10):
            qs[g % 3].dma_start(
                FLT[:, g * 8:(g + 1) * 8],
                STG[g * 4:(g + 1) * 4, :].rearrange("p (k a) -> p k a", a=8))

        # ---- batched row/col factors + outer products + banded matmuls ----
        I28b = I28[:].unsqueeze(1).broadcast_to([128, 10, 28])

        def fb(idx):
            return FLT[:, idx::8].unsqueeze(2).broadcast_to([128, 10, 28])

        EQ0 = pool.tile([128, 10 * 28], f32)
        EQ0v = EQ0[:].rearrange("p (g a) -> p g a", a=28)
        nv.tensor_tensor(out=EQ0v, in0=I28b, in1=fb(0), op=ALU.is_equal)
        RR = pool.tile([128, 10 * 28], f32)
        RRv = RR[:].rearrange("p (g a) -> p g a", a=28)
        nv.tensor_tensor(out=RRv, in0=EQ0v, in1=fb(2), op=ALU.mult)
        EQ1 = pool.tile([128, 10 * 28], f32)
        EQ1v = EQ1[:].rearrange("p (g a) -> p g a", a=28)
        nv.tensor_tensor(out=EQ1v, in0=I28b, in1=fb(1), op=ALU.is_equal)
        nv.tensor_tensor(out=EQ1v, in0=EQ1v, in1=fb(3), op=ALU.mult)
        nv.tensor_tensor(out=RR[:], in0=RR[:], in1=EQ1[:], op=ALU.add)

        EQ2 = pool.tile([128, 10 * 28], f32)
        EQ2v = EQ2[:].rearrange("p (g a) -> p g a", a=28)
        nv.tensor_tensor(out=EQ2v, in0=I28b, in1=fb(4), op=ALU.is_equal)
        CC = pool.tile([128, 10 * 28], f32)
        CCv = CC[:].rearrange("p (g a) -> p g a", a=28)
        nv.tensor_tensor(out=CCv, in0=EQ2v, in1=fb(6), op=ALU.mult)
        EQ3 = pool.tile([128, 10 * 28], f32)
        EQ3v = EQ3[:].rearrange("p (g a) -> p g a", a=28)
        nv.tensor_tensor(out=EQ3v, in0=I28b, in1=fb(5), op=ALU.is_equal)
        nv.tensor_tensor(out=EQ3v, in0=EQ3v, in1=fb(7), op=ALU.mult)
        nv.tensor_tensor(out=CC[:], in0=CC[:], in1=EQ3[:], op=ALU.add)

        G = pool.tile([P, 784], f32)
        GpsA = psum.tile([P, 392], f32)
        GpsB = psum.tile([P, 392], f32)
        for g in range(10):
            RHS = pool.tile([128, 784], bf16, tag="rhs", bufs=2)
            nv.tensor_tensor(out=RHS[:].rearrange("p (a b) -> p a b", b=28),
                             in0=RR[:, g * 28:(g + 1) * 28].unsqueeze(2)
                             .broadcast_to([128, 28, 28]),
                             in1=CC[:, g * 28:(g + 1) * 28].unsqueeze(1)
                             .broadcast_to([128, 28, 28]),
                             op=ALU.mult)
            lhsT_g = MMBH[:, 36 - 4 * g:76 - 4 * g]
            nc.tensor.matmul(GpsA[:], lhsT=lhsT_g, rhs=RHS[:, 0:392],
                             start=(g == 0), stop=(g == 9))
            nc.tensor.matmul(GpsB[:], lhsT=lhsT_g, rhs=RHS[:, 392:784],
                             start=(g == 0), stop=(g == 9))
        ns.copy(G[:, 0:392], GpsA[:])
        ns.copy(G[:, 392:784], GpsB[:])

        # ---- A sum over 4 row-blocks, counts, coef select ----
        A = pool.tile([P, 784], f32)
        S01 = pool.tile([P, 784], f32)
        nv.tensor_tensor(out=S01[:], in0=AIN[:, 0:784], in1=AIN[:, 784:1568],
                         op=ALU.add)
        nv.tensor_tensor(out=A[:], in0=AIN[:, 1568:2352], in1=AIN[:, 2352:3136],
                         op=ALU.add)
        nv.tensor_tensor(out=A[:], in0=A[:], in1=S01[:], op=ALU.add)
        CNT = pool.tile([P, 1], f32)
        nv.tensor_reduce(out=CNT[:], in_=A[:], axis=AX.X, op=ALU.add)
        ISZ = pool.tile([P, 1], u32)
        ts(ISZ[:], CNT[:], 0.5, None, ALU.is_lt)
        DEN = pool.tile([P, 1], f32)
        ts(DEN[:], CNT[:], 1.0, None, ALU.max)
        RDEN = pool.tile([P, 1], f32)
        nv.reciprocal(RDEN[:], DEN[:])
        AMN = pool.tile([P, 784], f32)
        ts(AMN[:], A[:], RDEN[:], None, ALU.mult)
        COEF = pool.tile([P, 784], f32)
        nv.select(COEF[:], ISZ[:].broadcast_to([P, 784]), G[:], AMN[:])
        ts(COEF[:], COEF[:], LAB[:], None, ALU.mult)

        # ---- coef^T chunks + fsm^T ----
        CT = pool.tile([RB, 7 * P], f32)
        for u in range(7):
            TPS = psum.tile([RB, P], f32, tag="tps", bufs=2)
            nc.tensor.transpose(TPS[:], COEF[:, u * RB:(u + 1) * RB], IDN[:P, :P])
            ns.copy(CT[:, u * P:(u + 1) * P], TPS[:])

        # FSMT[d, (dh, b, c)] chunks: acc over 7 s-chunks
        FSMT = pool.tile([128, 2 * P], f32)
        for dh in range(2):
            for b2 in range(B):
                fps = psum.tile([128, C], f32, tag="fsps", bufs=2)
                for u in range(7):
                    nc.tensor.matmul(
                        fps[:],
                        lhsT=FM[:, u * (B * D) + b2 * D + dh * 128:
                                u * (B * D) + b2 * D + (dh + 1) * 128],
                        rhs=CT[:, u * P + b2 * C:u * P + (b2 + 1) * C],
                        start=(u == 0), stop=(u == 6))
                ns.copy(FSMT[:, dh * P + b2 * C:dh * P + (b2 + 1) * C], fps[:])

        # ---- prescan: fsm norms (both b), softmax/term (both b) ----
        SQ = pool.tile([128, 2 * P], f32)
        nv.tensor_tensor(out=SQ[:], in0=FSMT[:], in1=FSMT[:], op=ALU.mult)
        ONC = pool.tile([128, 1], f32)
        nv.memset(ONC[:], 1.0)
        NNps = psum.tile([1, P], f32, tag="mm", bufs=2)
        for dh in range(2):
            nc.tensor.matmul(NNps[:], lhsT=ONC[:], rhs=SQ[:, dh * P:(dh + 1) * P],
                             start=(dh == 0), stop=(dh == 1))
        NN2 = pool.tile([1, P], f32)
        ts(NN2[:], NNps[:], 1e-24, None, ALU.max)
        NRM = pool.tile([1, P], f32)
        ns.activation(NRM[:], NN2[:], AFT.Sqrt)
        RN = pool.tile([1, P], f32)
        nv.reciprocal(RN[:], NRM[:])
        RNps = psum.tile([128, P], f32, tag="mm", bufs=2)
        nc.tensor.matmul(RNps[:], lhsT=ONR[:], rhs=RN[:], start=True, stop=True)
        FSMNT = pool.tile([128, 2 * P], f32)
        FSMNTv = FSMNT[:].rearrange("d (h p) -> d h p", p=P)
        nv.tensor_tensor(out=FSMNTv, in0=FSMT[:].rearrange("d (h p) -> d h p", p=P),
                         in1=RNps[:].unsqueeze(1).broadcast_to([128, 2, P]),
                         op=ALU.mult)

        # softmax/log terms for both b at once: LOG [C, (b, c')]
        TERM = pool.tile([C, B], f32)
        PR = pool.tile([C, 5], f32)
        for b2 in range(B):
            LOGps = psum.tile([C, C], f32, tag="mm", bufs=2)
            for dh in range(2):
                nc.tensor.matmul(LOGps[:],
                                 lhsT=FSMT[:, dh * P + b2 * C:dh * P + (b2 + 1) * C],
                                 rhs=PJT[:, dh * C:(dh + 1) * C],
                                 start=(dh == 0), stop=(dh == 1))
            MX = pool.tile([C, 1], f32, tag=f"mx{b2}")
            nv.tensor_reduce(out=MX[:], in_=LOGps[:], axis=AX.X, op=ALU.max)
            XT = pool.tile([C, C], f32, tag=f"xt{b2}")
            ts(XT[:], LOGps[:], MX[:], None, ALU.subtract)
            ET = pool.tile([C, C], f32, tag=f"et{b2}")
            ns.activation(ET[:], XT[:], AFT.Exp)
            SM = pool.tile([C, 1], f32, tag=f"sm{b2}")
            nv.tensor_reduce(out=SM[:], in_=ET[:], axis=AX.X, op=ALU.add)
            LGS = pool.tile([C, 1], f32, tag=f"lgs{b2}")
            ns.activation(LGS[:], SM[:], AFT.Ln)
            LGP = pool.tile([C, C], f32, tag=f"lgp{b2}")
            ts(LGP[:], XT[:], LGS[:], -100.0, ALU.subtract, ALU.max)
            SME = pool.tile([C, C], f32, tag=f"sme{b2}")
            nv.tensor_tensor(out=SME[:], in0=SM[:].broadcast_to([C, C]), in1=ET[:],
                             op=ALU.subtract)
            LSME = pool.tile([C, C], f32, tag=f"lsme{b2}")
            ns.activation(LSME[:], SME[:], AFT.Ln)
            L1P = pool.tile([C, C], f32, tag=f"l1p{b2}")
            ts(L1P[:], LSME[:], LGS[:], -100.0, ALU.subtract, ALU.max)
            DD = pool.tile([C, C], f32, tag=f"dd{b2}")
            nv.tensor_tensor(out=DD[:], in0=LGP[:], in1=L1P[:], op=ALU.subtract)
            DDS = pool.tile([C, C], f32, tag=f"dds{b2}")
            DDG = pool.tile([C, 1], f32, tag=f"ddg{b2}")
            nv.tensor_tensor(out=DDS[:], in0=EYE[:], in1=DD[:], op=ALU.mult)
            nv.tensor_reduce(out=DDG[:], in_=DDS[:], axis=AX.X, op=ALU.add)
            RSM = pool.tile([C, 1], f32, tag=f"rsm{b2}")
            nv.tensor_reduce(out=RSM[:], in_=L1P[:], axis=AX.X, op=ALU.add)
            TRM = pool.tile([C, 1], f32, tag=f"trm{b2}")
            nv.tensor_tensor(out=TRM[:], in0=DDG[:], in1=RSM[:], op=ALU.add)
            ts(TERM[:, b2:b2 + 1], TRM[:], -1.0 / C, None, ALU.mult)

        # PR col0 = pres0*term0, col1 = pres0 (iter-0 under fc0 == 0)
        nv.tensor_tensor(out=PR[:, 0:1], in0=TERM[:, 0:1], in1=LAB2[:, 0:1],
                         op=ALU.mult)
        nv.tensor_copy(PR[:, 1:2], LAB2[:, 0:1])

        # ---- iter 1 (fc after iter0 = 0.05 * pres0 * fsm0) ----
        FCNT = pool.tile([128, 2 * C], f32)
        FCNTv = FCNT[:].rearrange("d (h c) -> d h c", c=C)
        nv.tensor_tensor(out=FCNTv,
                         in0=FSMNT[:].rearrange("d (h p) -> d h p", p=P)[:, :, 0:C],
                         in1=LREP[:, 0:C].unsqueeze(1).broadcast_to([128, 2, C]),
                         op=ALU.mult)
        COSps = psum.tile([C, C], f32, tag="mm", bufs=2)
        for dh in range(2):
            nc.tensor.matmul(COSps[:],
                             lhsT=FSMNT[:, dh * P + C:dh * P + 2 * C],
                             rhs=FCNT[:, dh * C:(dh + 1) * C],
                             start=(dh == 0), stop=(dh == 1))
        COSC = pool.tile([C, C], f32)
        SGN = pool.tile([C, C], f32)
        ts(SGN[:], COSps[:], 0.0, None, ALU.is_lt)
        ts(SGN[:], SGN[:], -2.0, 1.0, ALU.mult, ALU.add)
        nv.tensor_tensor(out=COSC[:], in0=COSps[:], in1=SGN[:], op=ALU.mult)
        ts(COSC[:], COSC[:], 1e-5, 1.0 - 1e-5, ALU.max, ALU.min)
        LGC = pool.tile([C, C], f32)
        ns.activation(LGC[:], COSC[:], AFT.Ln)
        OM = pool.tile([C, C], f32)
        ts(OM[:], COSC[:], -1.0, 1.0, ALU.mult, ALU.add)
        LOM = pool.tile([C, C], f32)
        ns.activation(LOM[:], OM[:], AFT.Ln)
        DIF = pool.tile([C, C], f32)
        nv.tensor_tensor(out=DIF[:], in0=LGC[:], in1=LOM[:], op=ALU.subtract)
        IDM = pool.tile([C, C], f32)
        ts(IDM[:], EYE[:], LAB2[:, 1:2], None, ALU.mult)
        IDS = pool.tile([C, C], f32)
        IDG = pool.tile([C, 1], f32)
        nv.tensor_tensor(out=IDS[:], in0=IDM[:], in1=DIF[:], op=ALU.mult)
        nv.tensor_reduce(out=IDG[:], in_=IDS[:], axis=AX.X, op=ALU.add)
        R1 = pool.tile([C, 1], f32)
        nv.tensor_reduce(out=R1[:], in_=LOM[:], axis=AX.X, op=ALU.add)
        nv.tensor_tensor(out=PR[:, 4:5], in0=IDG[:], in1=R1[:], op=ALU.add)

        COSM = pool.tile([C, C], f32)
        nv.scalar_tensor_tensor(out=COSM[:], in0=EYE[:], scalar=-1e9, in1=COSC[:],
                                op0=ALU.mult, op1=ALU.add)
        OFF = pool.tile([C, 1], f32)
        nv.tensor_reduce(out=OFF[:], in_=COSM[:], axis=AX.X, op=ALU.max)
        QUAL = pool.tile([C, 1], f32)
        ts(QUAL[:], OFF[:], 0.6, None, ALU.is_lt)
        nv.tensor_tensor(out=QUAL[:], in0=QUAL[:], in1=LAB2[:, 1:2], op=ALU.mult)
        nv.tensor_copy(PR[:, 2:3], QUAL[:])
        nv.tensor_tensor(out=PR[:, 3:4], in0=QUAL[:], in1=TERM[:, 1:2], op=ALU.mult)

        # ---- final reduction + scalar assembly ----
        ONES20 = pool.tile([C, 1], f32)
        nv.memset(ONES20[:], 1.0)
        REDps = psum.tile([1, 5], f32, tag="mm", bufs=2)
        nc.tensor.matmul(REDps[:], lhsT=ONES20[:], rhs=PR[:], start=True, stop=True)
        RED = pool.tile([1, 5], f32)
        nv.tensor_copy(RED[:], REDps[:])
        # cols: 0 = S0, 1 = n0, 2 = n1, 3 = S1, 4 = ccf1_sum
        SCR = pool.tile([1, 6], f32)
        ts(SCR[:, 0:1], RED[:, 1:2], 1.0, None, ALU.max)        # max(n0,1)
        nv.reciprocal(SCR[:, 1:2], SCR[:, 0:1])
        nv.tensor_tensor(out=SCR[:, 2:3], in0=RED[:, 0:1], in1=SCR[:, 1:2],
                         op=ALU.mult)                            # l0
        nv.tensor_tensor(out=SCR[:, 2:3], in0=SCR[:, 2:3], in1=RED[:, 3:4],
                         op=ALU.add)                             # l0 + S1
        ts(SCR[:, 3:4], RED[:, 2:3], 1.0, None, ALU.max)        # max(n1,1)
        nv.reciprocal(SCR[:, 4:5], SCR[:, 3:4])
        nv.tensor_tensor(out=SCR[:, 2:3], in0=SCR[:, 2:3], in1=SCR[:, 4:5],
                         op=ALU.mult)                            # loss_cls
        ts(SCR[:, 5:6], RED[:, 1:2], ALPHA, BETA, ALU.mult, ALU.add)  # ccf0
        nv.tensor_tensor(out=SCR[:, 2:3], in0=SCR[:, 2:3], in1=SCR[:, 5:6],
                         op=ALU.add)
        OUT = pool.tile([1, 1], f32)
        ts(OUT[:], RED[:, 4:5], -1.0 / (C * C), None, ALU.mult)
        nv.tensor_tensor(out=OUT[:], in0=OUT[:], in1=SCR[:, 2:3], op=ALU.add)
        nc.sync.dma_start(o_loss, OUT[:])

    nc.compile()
    return nc


# --------------------------------------------------------------------------
# Host marshaling + driver
# --------------------------------------------------------------------------

_CACHE = {}


def _get_programs(hig, low, bg, CP):
    key = (float(hig), float(low), float(bg), CP)
    if key not in _CACHE:
        _CACHE[key] = (_build_a(hig, low, bg, CP), _build_b(CP))
    return _CACHE[key]


def _marshal_a(cam, CP, idxs):
    TOKS = (CP + 3) // 4
    T4 = 4 * TOKS
    grps = []
    t = T4
    while t > 0:
        grps.append(min(8, t))
        t -= grps[-1]
    wct = np.ascontiguousarray(
        W1D.reshape(4, RB, 28).transpose(1, 0, 2).reshape(RB, 4 * 28))
    bbs1 = np.zeros((T4, 16, 1), np.float32)
    for tok in range(T4):
        bbs1[tok] = float((tok % 4) * NPIX)
    selm = np.zeros((TOKS, 16 * T4), np.float32)
    for tok in range(T4):
        selm[tok // 4, 16 * tok:16 * (tok + 1)] = 1.0
    in_maps = []
    for core in range(8):
        b, q = core // NBLK, core % NBLK
        idx = idxs[b]
        camv = np.zeros((CP, NPIX), np.float32)
        if len(idx):
            camv[:len(idx)] = cam[b, idx, q * RB:(q + 1) * RB, :].reshape(
                len(idx), NPIX)
        tsl = idx[TOKS * q:TOKS * q + TOKS]
        # round-1 tokens: (class ct, blk) class-major, each [16, 3136]
        camt = np.zeros((T4, NPIX), np.float32)
        for t2, c in enumerate(tsl):
            camt[4 * t2:4 * (t2 + 1)] = cam[b, c].reshape(4, NPIX)
        m = {
            "camv": camv,
            "selm": selm,
            "idna": np.tile(np.eye(64, dtype=np.float32), (2, 1)),
            "wrt": np.ascontiguousarray(W1D[q * RB:(q + 1) * RB, :]),
            "wct": wct,
        }
        row0 = 0
        for i, g in enumerate(grps):
            m[f"camt{i}"] = np.ascontiguousarray(
                camt[row0:row0 + g]).reshape(16 * g, NPIX // 16)
            m[f"bbs1_{i}"] = np.ascontiguousarray(
                bbs1[row0:row0 + g]).reshape(16 * g, 1)
            row0 += g
        in_maps.append(m)
    return in_maps


def _marshal_b(res_a, fmap, cls_label, proj_weight, CP, idxs):
    P = B * C
    TOKS = (CP + 3) // 4
    # A partials: o_a[core] is [28, CP*28]; scatter slots -> classes,
    # block-major layout [P, NBLK*784]
    ain = np.zeros((P, NBLK * 784), np.float32)
    for core in range(8):
        b, q = core // NBLK, core % NBLK
        a = res_a[core]["o_a"].reshape(28, CP, 28)
        for j, c in enumerate(idxs[b]):
            ain[b * C + c, q * 784:(q + 1) * 784] = np.ascontiguousarray(
                a[:, j, :]).reshape(784)

    cdh = np.zeros((P, NC), np.float32)
    cdw = np.zeros((P, NC), np.float32)
    for core in range(8):
        b, q = core // NBLK, core % NBLK
        ch = res_a[core]["o_ch"].reshape(TOKS, 16, 2)
        cw = res_a[core]["o_cw"].reshape(TOKS, 16, 2)
        tsl = idxs[b][TOKS * q:TOKS * q + TOKS]
        for t, c in enumerate(tsl):
            # sparse_gather compaction order: slot i = f*16 + p
            cdh[b * C + c] = ch[t].T.reshape(NC)
            cdw[b * C + c] = cw[t].T.reshape(NC)

    rnk = np.zeros((P, NC), np.float32)
    rnk[:, :K_TOP] = 1.0 / K_TOP

    fm = np.asarray(fmap, np.float32).reshape(B, D, 7, 112)
    fmi = np.ascontiguousarray(fm.transpose(3, 2, 0, 1)).reshape(112, 7 * B * D)

    labf = np.asarray(cls_label, np.float32)
    return {
        "ain": ain,
        "cdh": cdh,
        "cdw": cdw,
        "rnk": rnk,
        "fmi": fmi,
        "prj": np.ascontiguousarray(
            np.asarray(proj_weight, np.float32).T.reshape(2, 128, C)
            .transpose(1, 0, 2)).reshape(128, 2 * C),
        "lab": labf.reshape(P, 1),
        "lab2": np.ascontiguousarray(labf.T),
        "lrep": np.tile(labf.reshape(1, P), (128, 1)),
        "eye": np.eye(C, dtype=np.float32),
        "i28": np.tile(np.arange(28, dtype=np.float32)[None, :], (128, 1)),
        "mmb": (np.arange(128)[:, None] // NC ==
                np.arange(76)[None, :] - 36).astype(np.float32),
        "idn": np.eye(128, dtype=np.float32),
        "onr": np.ones((1, 128), np.float32),
    }


LAST_EXEC_NS = {}
LAST_RES = {}


def _run(nc, in_maps, core_ids, tag="k"):
    if os.environ.get("BASSK_SIM") == "1":
        from concourse.bass_interp import CoreSim, MultiCoreSim
        if len(core_ids) == 1:
            sim = CoreSim(nc, trace=False, require_finite=False)
            sims = [sim]
        else:
            msim = MultiCoreSim(nc, num_cores=len(core_ids), trace=False,
                                require_finite=False)
            sims = [msim.cores[i] for i in core_ids]
            sim = msim
        for s, m in zip(sims, in_maps):
            for name, arr in m.items():
                s.tensor(name)[:] = arr
        sim.simulate(check_with_hw=False)
        outs = []
        for s in sims:
            d = {}
            for alloc in nc.m.functions[0].allocations:
                if getattr(alloc, "kind", None) == "ExternalOutput":
                    nm = alloc.memorylocations[0].name
                    d[nm] = np.array(s.tensor(nm))
            outs.append(d)
        return outs
    trace = os.environ.get("BASSK_TRACE") == "1"
    if trace:
        try:
            from antenv.axon_hooks import get_axon_ntff_profile_hook  # noqa: F401
        except Exception:
            trace = False
    res = run_bass_kernel_spmd(nc, in_maps, core_ids, trace=trace)
    if res.exec_time_ns is not None:
        LAST_EXEC_NS[tag] = res.exec_time_ns
    LAST_RES[tag] = res
    return res.results


def kernel(fmap, cam, cls_label, proj_weight, feature_contrast,
           hig_thre, low_thre, bg_thre):
    fmap = np.asarray(fmap, np.float32)
    cam = np.asarray(cam, np.float32)
    lab = np.asarray(cls_label, np.float32)
    fc0 = np.asarray(feature_contrast, np.float32)
    if np.any(fc0):
        raise NotImplementedError("kernel specialized to feature_contrast == 0")
    idxs = [np.where(lab[b] > 0.5)[0] for b in range(B)]
    cp_act = max(len(i) for i in idxs)
    CP = min(C, max(4, cp_act))
    nca, ncb = _get_programs(float(hig_thre), float(low_thre), float(bg_thre), CP)

    res_a = _run(nca, _marshal_a(cam, CP, idxs), list(range(8)), tag="A")
    in_b = _marshal_b(res_a, fmap, cls_label, proj_weight, CP, idxs)
    res_b = _run(ncb, [in_b], [0], tag="B")
    loss = np.float32(res_b[0]["o_loss"].reshape(-1)[0])
    return np.asarray(loss, dtype=np.float32).reshape(())
